# revision 46
# baseline (speedup 1.0000x reference)
"""Trainium2 8-core kernel for RMSNorm -> QKV -> RoPE -> causal SDPA -> out-proj.

Sharding: core c = b*4 + g handles batch b (of 2) and heads 4g..4g+3 (of 16).
Each core computes a partial out-projection [dim, tokens]; the host sums the
4 head-group partials per batch (the tensor-parallel "unshard") and adds b_o.

Cost-model-driven layout (TimelineSim charges matmuls by OUTPUT FREE SIZE
only — contraction depth and output partitions are free):
  - scores per (head, kb): [key 128, q free] trimmed to the causal triangle.
  - AV runs TRANSPOSED: out [q 128, d 65] so each accumulation step costs 65
    rows instead of ~512; the ones column gives the softmax denominator.
  - The normalized token-major AV result is returned to feature-major layout
    with DMA-engine transposes (14ns/32x32 tile, off the compute engines).
  - exp for a head PAIR is fused into one Activation instruction (the two
    heads' score tiles sit in adjacent PSUM banks).
  - r = rsqrt(mean x^2) rides into Q via r-scaled RoPE tables, into scores
    via the per-key `scale` operand of exp, and into V via a per-partition
    tensor_scalar during the PSUM->SBUF copy. r_tok (token-major r) comes
    from 16 free PE transposes of the r row.
  - PE is kept continuously busy (the cost model halves PE speed after any
    idle gap until 3us of continuous execution): the K projection starts at
    xT chunk 2 so the DMA stream stays ahead of the PE stream.
"""

import os

import numpy as np
import ml_dtypes

BF16 = ml_dtypes.bfloat16

DIM = 1024
HEADS = 16
DIM_HEAD = 64
T = 2048  # tokens per batch
B = 2
HPC = 4  # heads per core
F = HPC * DIM_HEAD  # 256 per-core head width
KC = DIM // 128  # 8 contraction chunks
KORD = [2, 3, 4, 5, 6, 7, 0, 1]  # kc order: first matmul waits for chunk 2

_NC_CACHE = {}


def _build_nc():
    import concourse.bacc as bacc
    import concourse.mybir as mybir
    import concourse.tile as tile
    from contextlib import ExitStack

    f32 = mybir.dt.float32
    bf16 = mybir.dt.bfloat16
    nc = bacc.Bacc()

    xT = nc.declare_dram_parameter("xT", [DIM, T], bf16, isOutput=False)
    wq = nc.declare_dram_parameter("wq", [DIM, F], bf16, isOutput=False)
    wk = nc.declare_dram_parameter("wk", [DIM, F], bf16, isOutput=False)
    wv = nc.declare_dram_parameter("wv", [DIM, F], bf16, isOutput=False)
    wo = nc.declare_dram_parameter("wo", [F, DIM], bf16, isOutput=False)
    cosT = nc.declare_dram_parameter("cosT", [128, T], bf16, isOutput=False)
    sinT = nc.declare_dram_parameter("sinT", [128, T], bf16, isOutput=False)
    perm = nc.declare_dram_parameter("perm", [128, 128], bf16, isOutput=False)
    masks = nc.declare_dram_parameter("masks", [128, 128], bf16, isOutput=False)
    ident = nc.declare_dram_parameter("ident", [128, 128], bf16, isOutput=False)
    out = nc.declare_dram_parameter("out", [DIM, T], bf16, isOutput=True)
    tap = os.environ.get("KTAP", "")
    dbg = None
    if tap:
        _tap_shapes = {
            "rtok": ([128, 16], f32),
            "qk": ([128, 4, T], bf16),
            "v": ([128, 16, HPC, 65], bf16),
            "avtok": ([128, 16, F], bf16),
            "avall": ([128, 2, T], bf16),
        }
        shp, dt = _tap_shapes[tap]
        dbg = nc.declare_dram_parameter("dbg", shp, dt, isOutput=True)

    Exp = mybir.ActivationFunctionType.Exp
    Sqrt = mybir.ActivationFunctionType.Sqrt
    mult = mybir.AluOpType.mult
    add = mybir.AluOpType.add

    with ExitStack() as ctx:
        tc = ctx.enter_context(tile.TileContext(nc))
        consts = ctx.enter_context(tc.tile_pool(name="consts", bufs=1))
        persist = ctx.enter_context(tc.tile_pool(name="persist", bufs=1))
        work = ctx.enter_context(tc.tile_pool(name="work", bufs=4))
        vecs = ctx.enter_context(tc.tile_pool(name="vecs", bufs=1))

        # ---- constants / inputs ----
        wk_sb = consts.tile([128, KC, F], bf16, tag="wk")
        wq_sb = consts.tile([128, KC, F], bf16, tag="wq")
        wv_sb = consts.tile([128, KC, F], bf16, tag="wv")
        wo_sb = consts.tile([128, 2, DIM], bf16, tag="wo")
        cos_sb = consts.tile([128, T], bf16, tag="cos")
        sin_sb = consts.tile([128, T], bf16, tag="sin")
        perm_sb = consts.tile([128, 128], bf16, tag="perm")
        mask_sb = consts.tile([128, 128], bf16, tag="mask")
        id_sb = consts.tile([128, 128], bf16, tag="ident")
        ones_col = consts.tile([128, 1], bf16, tag="onesc")
        one_f32 = consts.tile([1, 1], f32, tag="onef")
        xT_sb = persist.tile([128, KC, T], bf16, tag="xT")
        xT_r = xT.rearrange("(kc p) t -> p kc t", p=128)
        # wk first (first PE consumer), then xT chunks in consumption order
        # with the other weights slotted behind the early chunks
        nc.sync.dma_start(wk_sb, wk.rearrange("(kc p) f -> p kc f", p=128))
        for kc in KORD[:4]:
            nc.sync.dma_start(xT_sb[:, kc], xT_r[:, kc])
        nc.sync.dma_start(perm_sb, perm[:, :])
        nc.sync.dma_start(cos_sb, cosT[:, :])
        nc.sync.dma_start(sin_sb, sinT[:, :])
        for kc in KORD[4:]:
            nc.sync.dma_start(xT_sb[:, kc], xT_r[:, kc])
        nc.sync.dma_start(wq_sb, wq.rearrange("(kc p) f -> p kc f", p=128))
        nc.sync.dma_start(wv_sb, wv.rearrange("(kc p) f -> p kc f", p=128))
        nc.sync.dma_start(mask_sb, masks[:, :])
        nc.sync.dma_start(id_sb, ident[:, :])
        nc.sync.dma_start(wo_sb, wo.rearrange("(fc p) d -> p fc d", p=128))
        nc.vector.memset(ones_col, 1.0)
        nc.vector.memset(one_f32, 1.0)

        # persistent activations
        qk_sb = persist.tile([128, 4, T], bf16, tag="qk")  # 0,1=q fc0/1; 2,3=k
        v_sb = persist.tile([128, 16, HPC, 65], bf16, tag="v")
        av_tok = persist.tile([128, 16, F], bf16, tag="avtok")
        av_all = persist.tile([128, 2, T], bf16, tag="av")
        r_sb = vecs.tile([1, T], f32, tag="r")
        r_tok = vecs.tile([128, 16], f32, tag="rtok")
        r_bc = persist.tile([128, T], f32, tag="rbc")
        cosr_sb = persist.tile([128, T], bf16, tag="cosr")
        sinr_sb = persist.tile([128, T], bf16, tag="sinr")
        nc.vector.memset(v_sb[:, :, :, 64:65], 1.0)

        ctxA = ExitStack()
        psKQ = ctxA.enter_context(tc.tile_pool(name="psKQ", bufs=8, space="PSUM"))
        sbA = ctxA.enter_context(tc.tile_pool(name="sbA", bufs=1))
        xsq_sb = sbA.tile([128, KC, T], bf16, tag="xsq")

        # x^2 per chunk (DVE, chases the xT DMAs)
        for kc in KORD:
            nc.vector.tensor_mul(xsq_sb[:, kc], xT_sb[:, kc], xT_sb[:, kc])

        def proj_rope(fidx, psum_tiles, is_q):
            """Finish a Q/K projection: PSUM->SBUF copy, rotate-half perm
            matmul, rope multiply-adds into qk_sb[fidx]. Q uses the r-scaled
            tables so r_q rides in for free."""
            cc = cosr_sb if is_q else cos_sb
            ssb = sinr_sb if is_q else sin_sb
            for tt in range(4):
                ts = slice(tt * 512, (tt + 1) * 512)
                raw = work.tile([128, 512], bf16, tag="raw")
                if is_q or tt % 2 == 0:
                    nc.scalar.copy(out=raw, in_=psum_tiles[tt])
                else:
                    nc.vector.tensor_copy(out=raw, in_=psum_tiles[tt])
                pp = psKQ.tile([128, 512], f32, tag="proj", name=f"pp_{fidx}_{tt}")
                nc.tensor.matmul(pp, lhsT=perm_sb, rhs=raw, start=True, stop=True)
                t1 = work.tile([128, 512], bf16, tag="t1")
                nc.vector.tensor_tensor(t1, pp, ssb[:, ts], mult)
                t2 = work.tile([128, 512], bf16, tag="t2")
                nc.vector.tensor_tensor(t2, raw, cc[:, ts], mult)
                if is_q:
                    nc.vector.tensor_tensor(qk_sb[:, fidx, ts], t2, t1, add)
                else:
                    nc.gpsimd.tensor_tensor(qk_sb[:, fidx, ts], t2, t1, add)

        # ---- K projection (both fc), chunk-paced off the xT DMA stream ----
        psK = {}
        for fc in range(2):
            for tt in range(4):
                psK[(fc, tt)] = psKQ.tile(
                    [128, 512], f32, tag="proj", name=f"k_{fc}_{tt}"
                )
        # chunk-paced sweep for the first 6 chunks, then finish per-tile so
        # the PSUM->SBUF copies drain early (frees the ring for ss/Q)
        for kc in KORD[:6]:
            for fc in range(2):
                for tt in range(4):
                    nc.tensor.matmul(
                        psK[(fc, tt)],
                        lhsT=wk_sb[:, kc, fc * 128 : (fc + 1) * 128],
                        rhs=xT_sb[:, kc, tt * 512 : (tt + 1) * 512],
                        start=(kc == KORD[0]),
                        stop=False,
                    )
        for fc in range(2):
            for tt in range(4):
                for kc in KORD[6:]:
                    nc.tensor.matmul(
                        psK[(fc, tt)],
                        lhsT=wk_sb[:, kc, fc * 128 : (fc + 1) * 128],
                        rhs=xT_sb[:, kc, tt * 512 : (tt + 1) * 512],
                        start=False,
                        stop=(kc == KORD[-1]),
                    )
        for fc in range(2):
            proj_rope(2 + fc, [psK[(fc, tt)] for tt in range(4)], False)

        # ---- sum(x^2) ones-matmuls with the r-chain pipelined per slice ----
        ss_sb = sbA.tile([1, T], f32, tag="ss")
        sq_sb = sbA.tile([1, T], f32, tag="sq")
        for s in range(4):
            ts = slice(s * 512, (s + 1) * 512)
            ss_ps = psKQ.tile([1, 512], f32, tag="proj", name=f"ss_{s}")
            for kc in range(KC):
                nc.tensor.matmul(
                    ss_ps,
                    lhsT=ones_col,
                    rhs=xsq_sb[:, kc, s * 512 : (s + 1) * 512],
                    start=(kc == 0),
                    stop=(kc == KC - 1),
                )
            nc.scalar.copy(out=ss_sb[:, ts], in_=ss_ps)
            nc.scalar.activation(sq_sb[:, ts], ss_sb[:, ts], Sqrt, scale=1.0 / DIM)
            nc.vector.reciprocal(r_sb[:, ts], sq_sb[:, ts])
            nc.gpsimd.partition_broadcast(r_bc[:, ts], r_sb[:, ts])
            nc.gpsimd.tensor_tensor(cosr_sb[:, ts], cos_sb[:, ts], r_bc[:, ts], mult)
            nc.gpsimd.tensor_tensor(sinr_sb[:, ts], sin_sb[:, ts], r_bc[:, ts], mult)
        # Q fc0 projection
        psQ0 = [
            psKQ.tile([128, 512], f32, tag="proj", name=f"q0_{tt}")
            for tt in range(4)
        ]
        for kc in range(KC):
            for tt in range(4):
                nc.tensor.matmul(
                    psQ0[tt],
                    lhsT=wq_sb[:, kc, 0:128],
                    rhs=xT_sb[:, kc, tt * 512 : (tt + 1) * 512],
                    start=(kc == 0),
                    stop=(kc == KC - 1),
                )
        # r_tok via PE transposes of the r row
        rtok_ps = psKQ.tile([128, 16], f32, tag="proj", name="rtokps")
        for i in range(16):
            nc.tensor.transpose(
                rtok_ps[:, i : i + 1], r_sb[0:1, i * 128 : (i + 1) * 128],
                one_f32,
            )
        nc.vector.tensor_copy(out=r_tok, in_=rtok_ps)

        proj_rope(0, psQ0, True)

        # ---- Q fc1 ----
        psQ1 = [
            psKQ.tile([128, 512], f32, tag="proj", name=f"q1_{tt}")
            for tt in range(4)
        ]
        for kc in range(KC):
            for tt in range(4):
                nc.tensor.matmul(
                    psQ1[tt],
                    lhsT=wq_sb[:, kc, 128:256],
                    rhs=xT_sb[:, kc, tt * 512 : (tt + 1) * 512],
                    start=(kc == 0),
                    stop=(kc == KC - 1),
                )
        proj_rope(1, psQ1, True)

        # ---- V projection (token-major) + r_tok scaling ----
        ctxA.close()

        # ---- attention: scores [k,q] -> paired exp -> transposed AV ----
        # PSUM: sc ring (2x2 banks, also V-proj) + av4/po ring (3) + avT (1)
        with (
            tc.tile_pool(name="psSC", bufs=3, space="PSUM") as psSC,
            tc.tile_pool(name="psAV", bufs=2, space="PSUM") as psAV,
            tc.tile_pool(name="expp", bufs=6) as expp,
            tc.tile_pool(name="recp", bufs=4) as recp,
        ):
            def v_proj(tt):
                psv = psSC.tile([128, 256], f32, tag="sc", name=f"v_{tt}")
                for kc in range(KC):
                    nc.tensor.matmul(
                        psv,
                        lhsT=xT_sb[:, kc, tt * 128 : (tt + 1) * 128],
                        rhs=wv_sb[:, kc, :],
                        start=(kc == 0),
                        stop=(kc == KC - 1),
                    )
                nc.vector.tensor_scalar(
                    out=v_sb[:, tt, :, 0:64],
                    in0=psv.rearrange("p (h d) -> p h d", h=HPC),
                    scalar1=r_tok[:, tt : tt + 1],
                    scalar2=None,
                    op0=mult,
                )

            def emit_outproj_do(qtp, do):
                po = psSC.tile([128, 512], f32, tag="sc", name=f"o_{qtp}_{do}")
                for fc in range(2):
                    nc.tensor.matmul(
                        po,
                        lhsT=wo_sb[:, fc, do * 128 : (do + 1) * 128],
                        rhs=av_all[:, fc, qtp * 512 : (qtp + 1) * 512],
                        start=(fc == 0),
                        stop=(fc == 1),
                    )
                ob = work.tile([128, 512], bf16, tag="ob")
                nc.vector.tensor_copy(out=ob, in_=po)
                nc.sync.dma_start(
                    out.rearrange("(do p) t -> p do t", p=128)[
                        :, do, qtp * 512 : (qtp + 1) * 512
                    ],
                    ob,
                )

            pending_oq = None  # (qt_prev, next_do)
            for qt in range(4):
                q0 = qt * 512
                for pi in range(2):
                    # full-bank tiles: matmul start=True zeroes the whole 2KB
                    # bank, so only the FIRST write into each bank uses it
                    av4 = [
                        psAV.tile([128, 4, 128], f32, tag="av4",
                                  name=f"av_{qt}_{pi}_{x}")
                        for x in range(2)
                    ]
                    def emit_av(kb, ex):
                        for qbl in range(4):
                            qb = 4 * qt + qbl
                            if kb > qb:
                                continue
                            for x in range(2):
                                nc.tensor.matmul(
                                    av4[x][:, qbl, 0:65],
                                    lhsT=ex[:, x * 512 + qbl * 128 : x * 512 + (qbl + 1) * 128],
                                    rhs=v_sb[:, kb, 2 * pi + x, :],
                                    start=(kb == 0 and qbl == 0),
                                    stop=(kb == qb),
                                    skip_group_check=True,
                                )

                    nkb = 4 * qt + 4
                    pend = None
                    for kb in range(nkb + 1):
                        cur = None
                        if kb < nkb:
                            # stagger this quarter's V projections into pair
                            # 0's kb loop so PE fills gaps while Act runs exp
                            if pi == 0 and qt == 0:
                                v_proj(kb)
                            c0 = max(0, kb * 128 - q0)
                            sc = psSC.tile(
                                [128, 1024], f32, tag="sc",
                                name=f"sc_{qt}_{pi}_{kb}"
                            )
                            for x in range(2):
                                rX = slice(x * 64, x * 64 + 64)
                                nc.tensor.matmul(
                                    sc[:, x * 512 + c0 : x * 512 + 512],
                                    lhsT=qk_sb[rX, 2 + pi, kb * 128 : (kb + 1) * 128],
                                    rhs=qk_sb[rX, pi, q0 + c0 : q0 + 512],
                                    start=True,
                                    stop=True,
                                )
                            if pi == 0 and qt > 0 and kb < 4:
                                v_proj(4 * qt + kb)
                            # spread the previous quarter's out-projection
                            # over this kb loop to keep Act fed with scores
                            if pi == 0 and pending_oq is not None and kb >= 2:
                                qtp, nd = pending_oq
                                todo = 8 - nd
                                left = nkb - kb
                                n_emit = -(-todo // max(left, 1))
                                for _ in range(min(n_emit, todo)):
                                    emit_outproj_do(qtp, nd)
                                    nd += 1
                                pending_oq = (qtp, nd) if nd < 8 else None
                            ex = expp.tile([128, 1024], bf16, tag="exp")
                            if c0 == 0:
                                nc.scalar.activation(
                                    ex, sc, Exp, scale=r_tok[:, kb : kb + 1]
                                )
                            else:
                                for x in range(2):
                                    nc.scalar.activation(
                                        ex[:, x * 512 + c0 : x * 512 + 512],
                                        sc[:, x * 512 + c0 : x * 512 + 512],
                                        Exp,
                                        scale=r_tok[:, kb : kb + 1],
                                    )
                            if kb >= 4 * qt:  # diagonal block: causal mask
                                for x in range(2):
                                    nc.gpsimd.tensor_tensor(
                                        ex[:, x * 512 + c0 : x * 512 + c0 + 128],
                                        ex[:, x * 512 + c0 : x * 512 + c0 + 128],
                                        mask_sb,
                                        mult,
                                    )
                            cur = (kb, ex)
                        if pend is not None:
                            emit_av(*pend)
                        pend = cur
                    # normalize (rows 0..63 / row 64) into token-major av_tok
                    rec4s = []
                    for x in range(2):
                        rec4 = recp.tile([128, 4], f32, tag="rec")
                        nc.vector.reciprocal(rec4, av4[x][:, :, 64:65])
                        rec4s.append(rec4)
                    for qbl in range(4):
                        for x in range(2):
                            h = 2 * pi + x
                            nc.vector.tensor_scalar(
                                out=av_tok[:, 4 * qt + qbl, h * 64 : (h + 1) * 64],
                                in0=av4[x][:, qbl, 0:64],
                                scalar1=rec4s[x][:, qbl : qbl + 1],
                                scalar2=None,
                                op0=mult,
                            )
                        # last quarter: pipeline the tail at half-quarter
                        # granularity straight out of the norms
                        if qt == 3 and pi == 1 and qbl % 2 == 1:
                            hf = qbl // 2
                            ts0 = 3 * 512 + hf * 256
                            avT3 = psSC.tile(
                                [128, 4, 128], bf16, tag="sc", name=f"avt3_{hf}"
                            )
                            for j in range(2):
                                tt = 12 + hf * 2 + j
                                for fc in range(2):
                                    nc.tensor.transpose(
                                        avT3[:, fc * 2 + j, :],
                                        av_tok[:, tt, fc * 128 : (fc + 1) * 128],
                                        id_sb,
                                    )
                            for fc in range(2):
                                nc.vector.tensor_copy(
                                    out=av_all[:, fc, ts0 : ts0 + 256],
                                    in_=avT3[:, fc * 2 : fc * 2 + 2, :],
                                )
                            for do in range(8):
                                po = psSC.tile(
                                    [128, 256], f32, tag="sc",
                                    name=f"o3_{hf}_{do}"
                                )
                                for fc in range(2):
                                    nc.tensor.matmul(
                                        po,
                                        lhsT=wo_sb[:, fc, do * 128 : (do + 1) * 128],
                                        rhs=av_all[:, fc, ts0 : ts0 + 256],
                                        start=(fc == 0),
                                        stop=(fc == 1),
                                    )
                                ob = work.tile([128, 256], bf16, tag="ob")
                                nc.vector.tensor_copy(out=ob, in_=po)
                                nc.sync.dma_start(
                                    out.rearrange("(do p) t -> p do t", p=128)[
                                        :, do, ts0 : ts0 + 256
                                    ],
                                    ob,
                                )
                # back to feature-major via PE transposes (53ns each);
                # qt3 already handled inline at half-quarter granularity
                if qt < 3:
                    avT = psSC.tile(
                        [128, 8, 128], bf16, tag="sc", name=f"avt_{qt}"
                    )
                    for j, tt in enumerate(range(4 * qt, 4 * qt + 4)):
                        for fc in range(2):
                            nc.tensor.transpose(
                                avT[:, fc * 4 + j, :],
                                av_tok[:, tt, fc * 128 : (fc + 1) * 128],
                                id_sb,
                            )
                    for fc in range(2):
                        nc.vector.tensor_copy(
                            out=av_all[:, fc, q0 : q0 + 512],
                            in_=avT[:, fc * 4 : fc * 4 + 4, :],
                        )
                    # out-projection deferred into the next quarter's kb loop
                    pending_oq = (qt, 0)
            if tap == "rtok":
                nc.sync.dma_start(dbg[:, :], r_tok)
            elif tap == "qk":
                nc.sync.dma_start(dbg[:, :, :], qk_sb)
            elif tap == "v":
                nc.sync.dma_start(dbg[:, :, :, :], v_sb)
            elif tap == "avtok":
                nc.sync.dma_start(dbg[:, :, :], av_tok)
            elif tap == "avall":
                nc.sync.dma_start(dbg[:, :, :], av_all)
    nc.compile()
    return nc


def _host_inputs(x, norm_w, w_qkv, w_o, sin, cos):
    """Build the 8 per-core input maps (all bf16)."""
    n = T
    w_eff = np.asarray(w_qkv, np.float64) * np.asarray(norm_w, np.float64)[:, None]
    sin_n = np.asarray(sin, np.float32)[:n]  # [T, 64]
    cos_n = np.asarray(cos, np.float32)[:n]
    sign = np.concatenate([-np.ones(32, np.float32), np.ones(32, np.float32)])
    cos_tile = np.tile(cos_n.T, (2, 1))  # [128, T]
    sin_tile = np.tile((sin_n * sign[None, :]).T, (2, 1))  # [128, T]
    perm = np.zeros((128, 128), np.float32)
    for m in range(128):
        d = m % 64
        k = m + 32 if d < 32 else m - 32
        perm[k, m] = 1.0
    ident_np = np.eye(128, dtype=np.float32)
    ql = np.arange(128)[None, :]
    key = np.arange(128)[:, None]
    masks = (ql >= key).astype(np.float32)

    in_maps = []
    for c in range(8):
        b, g = c // 4, c % 4
        fs = slice(g * F, (g + 1) * F)
        in_maps.append(
            {
                "xT": np.ascontiguousarray(np.asarray(x, np.float32)[b].T).astype(BF16),
                "wq": (w_eff[:, 0:DIM][:, fs] * (DIM_HEAD ** -0.5)).astype(BF16),
                "wk": w_eff[:, DIM : 2 * DIM][:, fs].astype(BF16),
                "wv": w_eff[:, 2 * DIM : 3 * DIM][:, fs].astype(BF16),
                "wo": np.asarray(w_o, np.float32)[fs, :].astype(BF16),
                "cosT": cos_tile.astype(BF16),
                "sinT": sin_tile.astype(BF16),
                "perm": perm.astype(BF16),
                "masks": masks.astype(BF16),
                "ident": ident_np.astype(BF16),
            }
        )
    return in_maps


def kernel(x, norm_w, w_qkv, w_o, b_o, sin, cos):
    from concourse.bass_utils import run_bass_kernel_spmd

    if "nc" not in _NC_CACHE:
        _NC_CACHE["nc"] = _build_nc()
    nc = _NC_CACHE["nc"]
    in_maps = _host_inputs(x, norm_w, w_qkv, w_o, sin, cos)
    trace = bool(int(os.environ.get("KERNEL_TRACE", "0")))
    res = run_bass_kernel_spmd(nc, in_maps, core_ids=list(range(8)), trace=trace)
    if trace and res.exec_time_ns is not None:
        print(f"HW exec time: {res.exec_time_ns} ns")
    outs = [r["out"].astype(np.float32) for r in res.results]  # [1024, T] fm
    b_o = np.asarray(b_o, np.float32)
    full = np.empty((B, T, DIM), np.float32)
    for b in range(B):
        acc = outs[b * 4] + outs[b * 4 + 1] + outs[b * 4 + 2] + outs[b * 4 + 3]
        full[b] = acc.T + b_o[None, :]
    return full


# revision 68
# speedup vs baseline: 1.1103x; 1.1103x over previous
"""Trainium2 8-core kernel for RMSNorm -> QKV -> RoPE -> causal SDPA -> out-proj.

Sharding: core c = b*4 + g handles batch b (of 2) and heads 4g..4g+3 (of 16).
Each core computes a partial out-projection [dim, tokens]; the host sums the
4 head-group partials per batch (the tensor-parallel "unshard") and adds b_o.

Cost-model-driven layout (TimelineSim charges matmuls by OUTPUT FREE SIZE
only — contraction depth and output partitions are free):
  - scores per (head, kb): [key 128, q free] trimmed to the causal triangle.
  - AV runs TRANSPOSED: out [q 128, d 65] so each accumulation step costs 65
    rows instead of ~512; the ones column gives the softmax denominator.
  - The normalized token-major AV result is returned to feature-major layout
    with DMA-engine transposes (14ns/32x32 tile, off the compute engines).
  - exp for a head PAIR is fused into one Activation instruction (the two
    heads' score tiles sit in adjacent PSUM banks).
  - r = rsqrt(mean x^2) rides into Q via r-scaled RoPE tables, into scores
    via the per-key `scale` operand of exp, and into V via a per-partition
    tensor_scalar during the PSUM->SBUF copy. r_tok (token-major r) comes
    from 16 free PE transposes of the r row.
  - PE is kept continuously busy (the cost model halves PE speed after any
    idle gap until 3us of continuous execution): the K projection starts at
    xT chunk 2 so the DMA stream stays ahead of the PE stream.
"""

import os

import numpy as np
import ml_dtypes

BF16 = ml_dtypes.bfloat16

DIM = 1024
HEADS = 16
DIM_HEAD = 64
T = 2048  # tokens per batch
B = 2
HPC = 4  # heads per core
F = HPC * DIM_HEAD  # 256 per-core head width
KC = DIM // 128  # 8 contraction chunks
KORD = [2, 3, 4, 5, 6, 7, 0, 1]  # kc order: first matmul waits for chunk 2
TAIL_FINE = int(os.environ.get("KTAIL", "0"))
QT0_IN_RING = int(os.environ.get("KQT0", "0"))

_NC_CACHE = {}


def _build_nc():
    import concourse.bacc as bacc
    import concourse.mybir as mybir
    import concourse.tile as tile
    from contextlib import ExitStack

    f32 = mybir.dt.float32
    bf16 = mybir.dt.bfloat16
    nc = bacc.Bacc()

    xT = nc.declare_dram_parameter("xT", [DIM, T], bf16, isOutput=False)
    wq = nc.declare_dram_parameter("wq", [DIM, F], bf16, isOutput=False)
    wk = nc.declare_dram_parameter("wk", [DIM, F], bf16, isOutput=False)
    wv = nc.declare_dram_parameter("wv", [DIM, F], bf16, isOutput=False)
    wo = nc.declare_dram_parameter("wo", [F, DIM], bf16, isOutput=False)
    cosT = nc.declare_dram_parameter("cosT", [128, T], bf16, isOutput=False)
    sinT = nc.declare_dram_parameter("sinT", [128, T], bf16, isOutput=False)
    perm = nc.declare_dram_parameter("perm", [128, 128], bf16, isOutput=False)
    masks = nc.declare_dram_parameter("masks", [128, 128], bf16, isOutput=False)
    ident = nc.declare_dram_parameter("ident", [128, 128], bf16, isOutput=False)
    out = nc.declare_dram_parameter("out", [DIM, T], bf16, isOutput=True)
    tap = os.environ.get("KTAP", "")
    dbg = None
    if tap:
        _tap_shapes = {
            "rtok": ([128, 16], f32),
            "qk": ([128, 4, T], bf16),
            "v": ([128, 16, HPC, 65], bf16),
            "avtok": ([128, 16, F], bf16),
            "avall": ([128, 2, T], bf16),
        }
        shp, dt = _tap_shapes[tap]
        dbg = nc.declare_dram_parameter("dbg", shp, dt, isOutput=True)

    Exp = mybir.ActivationFunctionType.Exp
    Sqrt = mybir.ActivationFunctionType.Sqrt
    mult = mybir.AluOpType.mult
    add = mybir.AluOpType.add

    with ExitStack() as ctx:
        tc = ctx.enter_context(tile.TileContext(nc))
        consts = ctx.enter_context(tc.tile_pool(name="consts", bufs=1))
        persist = ctx.enter_context(tc.tile_pool(name="persist", bufs=1))
        work = ctx.enter_context(tc.tile_pool(name="work", bufs=4))
        vecs = ctx.enter_context(tc.tile_pool(name="vecs", bufs=1))

        # ---- constants / inputs ----
        wk_sb = consts.tile([128, KC, F], bf16, tag="wk")
        wq_sb = consts.tile([128, KC, F], bf16, tag="wq")
        wv_sb = consts.tile([128, KC, F], bf16, tag="wv")
        wo_sb = consts.tile([128, 2, DIM], bf16, tag="wo")
        cos_sb = consts.tile([128, T], bf16, tag="cos")
        sin_sb = consts.tile([128, T], bf16, tag="sin")
        perm_sb = consts.tile([128, 128], bf16, tag="perm")
        mask_sb = consts.tile([128, 128], bf16, tag="mask")
        id_sb = consts.tile([128, 128], bf16, tag="ident")
        ones_col = consts.tile([128, 1], bf16, tag="onesc")
        one_f32 = consts.tile([1, 1], f32, tag="onef")
        xT_sb = persist.tile([128, KC, T], bf16, tag="xT")
        xT_r = xT.rearrange("(kc p) t -> p kc t", p=128)
        # wk first (first PE consumer), then xT chunks in consumption order
        # with the other weights slotted behind the early chunks
        nc.sync.dma_start(wk_sb, wk.rearrange("(kc p) f -> p kc f", p=128))
        nc.sync.dma_start(wq_sb, wq.rearrange("(kc p) f -> p kc f", p=128))
        for kc in KORD[:4]:
            nc.sync.dma_start(xT_sb[:, kc], xT_r[:, kc])
        nc.sync.dma_start(perm_sb, perm[:, :])
        nc.sync.dma_start(cos_sb, cosT[:, :])
        nc.sync.dma_start(sin_sb, sinT[:, :])
        for kc in KORD[4:]:
            nc.sync.dma_start(xT_sb[:, kc], xT_r[:, kc])
        nc.sync.dma_start(wv_sb, wv.rearrange("(kc p) f -> p kc f", p=128))
        nc.sync.dma_start(mask_sb, masks[:, :])
        nc.sync.dma_start(id_sb, ident[:, :])
        nc.sync.dma_start(wo_sb, wo.rearrange("(fc p) d -> p fc d", p=128))
        nc.vector.memset(ones_col, 1.0)
        nc.vector.memset(one_f32, 1.0)

        # persistent activations
        qk_sb = persist.tile([128, 4, T], bf16, tag="qk")  # 0,1=q fc0/1; 2,3=k
        v_sb = persist.tile([128, 16, HPC, 65], bf16, tag="v")
        av_tok = persist.tile([128, 16, F], bf16, tag="avtok")
        av_all = persist.tile([128, 2, T], bf16, tag="av")
        r_sb = vecs.tile([1, T], f32, tag="r")
        r_tok = vecs.tile([128, 16], f32, tag="rtok")
        r_bc = persist.tile([128, T], f32, tag="rbc")
        cosr_sb = persist.tile([128, T], bf16, tag="cosr")
        sinr_sb = persist.tile([128, T], bf16, tag="sinr")
        qraw_sb = persist.tile([128, 2, 4, 512], bf16, tag="qraw")
        kraw_sb = persist.tile([128, 2, 4, 512], bf16, tag="kraw")
        nc.vector.memset(v_sb[:, :, :, 64:65], 1.0)
        # preload the Sqrt/Exp activation tables while DMAs stream in
        dum = vecs.tile([1, 1], f32, tag="dum")
        nc.scalar.activation(dum, one_f32, Sqrt)
        nc.scalar.activation(dum, dum, Exp)

        expp = ctx.enter_context(tc.tile_pool(name="expp", bufs=6))
        recp = ctx.enter_context(tc.tile_pool(name="recp", bufs=4))

        ctxA = ExitStack()
        psKQ = ctxA.enter_context(tc.tile_pool(name="psKQ", bufs=8, space="PSUM"))
        sbA = ctxA.enter_context(tc.tile_pool(name="sbA", bufs=1))
        xsq_sb = sbA.tile([128, KC, T], bf16, tag="xsq")

        # x^2 per chunk (DVE, chases the xT DMAs)
        for kc in KORD:
            nc.vector.tensor_mul(xsq_sb[:, kc], xT_sb[:, kc], xT_sb[:, kc])

        def rope_tt(fidx, tt, pool):
            """RoPE one 512-token slice of Q/K from the raw SBUF copy:
            rotate-half perm matmul + two multiplies + add into qk_sb.
            Q (fidx 0,1) uses the r-scaled tables so r_q rides in free."""
            ts = slice(tt * 512, (tt + 1) * 512)
            is_q = fidx < 2
            raw = (qraw_sb if is_q else kraw_sb)[:, fidx % 2, tt]
            cc = cosr_sb if is_q else cos_sb
            ssb = sinr_sb if is_q else sin_sb
            pp = pool.tile([128, 512], f32, tag="sc" if pool is not psKQ else "proj",
                           name=f"pp_{fidx}_{tt}")
            nc.tensor.matmul(pp, lhsT=perm_sb, rhs=raw, start=True, stop=True)
            t1 = work.tile([128, 512], bf16, tag="t1")
            nc.vector.tensor_tensor(t1, pp, ssb[:, ts], mult)
            t2 = work.tile([128, 512], bf16, tag="t2")
            nc.vector.tensor_tensor(t2, raw, cc[:, ts], mult)
            nc.vector.tensor_tensor(qk_sb[:, fidx, ts], t2, t1, add)

        # ---- wave 1: K-fc0 + Q-fc0 projections, chunk-paced off DMA ----
        psW = {}
        for nm in ("k0", "q0"):
            for tt in range(4):
                psW[(nm, tt)] = psKQ.tile(
                    [128, 512], f32, tag="proj", name=f"{nm}_{tt}"
                )
        for kc in KORD:
            for tt in range(4):
                ts = slice(tt * 512, (tt + 1) * 512)
                nc.tensor.matmul(
                    psW[("k0", tt)],
                    lhsT=wk_sb[:, kc, 0:128],
                    rhs=xT_sb[:, kc, ts],
                    start=(kc == KORD[0]),
                    stop=(kc == KORD[-1]),
                )
                nc.tensor.matmul(
                    psW[("q0", tt)],
                    lhsT=wq_sb[:, kc, 0:128],
                    rhs=xT_sb[:, kc, ts],
                    start=(kc == KORD[0]),
                    stop=(kc == KORD[-1]),
                )
        # free the k0 slots first (Act; DVE is still finishing x^2);
        # q0 copies are deferred into wave 2 so the r-chain starts sooner
        for tt in range(4):
            nc.scalar.copy(out=kraw_sb[:, 0, tt], in_=psW[("k0", tt)])

        # ---- wave 2: ss/r-chain + K-fc1 + Q-fc1 interleaved ----
        ss_sb = sbA.tile([1, T], f32, tag="ss")

        def proj_fc1(which, tt):
            w = wk_sb if which == "k" else wq_sb
            psq = psKQ.tile([128, 512], f32, tag="proj", name=f"{which}1_{tt}")
            for kc in range(KC):
                nc.tensor.matmul(
                    psq,
                    lhsT=w[:, kc, 128:256],
                    rhs=xT_sb[:, kc, tt * 512 : (tt + 1) * 512],
                    start=(kc == 0),
                    stop=(kc == KC - 1),
                )
            if which == "k":
                nc.vector.tensor_copy(out=kraw_sb[:, 1, tt], in_=psq)
            else:
                nc.scalar.copy(out=qraw_sb[:, 1, tt], in_=psq)

        def ss_slice(s):
            ts = slice(s * 512, (s + 1) * 512)
            ss_ps = psKQ.tile([1, 512], f32, tag="proj", name=f"ss_{s}")
            for kc in range(KC):
                nc.tensor.matmul(
                    ss_ps,
                    lhsT=ones_col,
                    rhs=xsq_sb[:, kc, s * 512 : (s + 1) * 512],
                    start=(kc == 0),
                    stop=(kc == KC - 1),
                )
            nc.scalar.activation(
                ss_sb[:, ts], ss_ps, Sqrt, scale=1.0 / DIM
            )
            nc.vector.reciprocal(r_sb[:, ts], ss_sb[:, ts])
            nc.gpsimd.partition_broadcast(r_bc[:, ts], r_sb[:, ts])
            nc.gpsimd.tensor_tensor(cosr_sb[:, ts], cos_sb[:, ts], r_bc[:, ts], mult)
            nc.gpsimd.tensor_tensor(sinr_sb[:, ts], sin_sb[:, ts], r_bc[:, ts], mult)

        proj_fc1("q", 0)
        ss_slice(0)
        for tt in (0, 1):
            nc.scalar.copy(out=qraw_sb[:, 0, tt], in_=psW[("q0", tt)])
        rope_tt(2, 0, psKQ)
        proj_fc1("k", 0)
        ss_slice(1)
        rope_tt(0, 0, psKQ)
        for tt in (2, 3):
            nc.scalar.copy(out=qraw_sb[:, 0, tt], in_=psW[("q0", tt)])
        proj_fc1("q", 1)
        rope_tt(3, 0, psKQ)
        proj_fc1("k", 1)
        rope_tt(1, 0, psKQ)
        ss_slice(2)
        proj_fc1("q", 2)
        proj_fc1("k", 2)
        ss_slice(3)
        proj_fc1("q", 3)
        proj_fc1("k", 3)
        # r_tok via PE transposes of the r row
        rtok_ps = psKQ.tile([128, 16], f32, tag="proj", name="rtokps")
        for i in range(16):
            nc.tensor.transpose(
                rtok_ps[:, i : i + 1], r_sb[0:1, i * 128 : (i + 1) * 128],
                one_f32,
            )
        nc.vector.tensor_copy(out=r_tok, in_=rtok_ps)

        # ---- attention: scores [k,q] -> paired exp -> transposed AV ----
        # Quarter 0 runs INSIDE the psKQ ring (overlapping the QKV tail);
        # quarters 1-3 use dedicated pools: sc ring (3x2 banks, also V/pp/
        # outproj/avT) + av4 ring (2 banks).
        state = {"pending_oq": None, "sc_pool": psKQ, "split_sc": True}

        def v_proj(tt):
            pool = state["sc_pool"]
            tg = "proj" if pool is psKQ else "sc"
            psv = pool.tile([128, 256], f32, tag=tg, name=f"v_{tt}")
            for kc in range(KC):
                nc.tensor.matmul(
                    psv,
                    lhsT=xT_sb[:, kc, tt * 128 : (tt + 1) * 128],
                    rhs=wv_sb[:, kc, :],
                    start=(kc == 0),
                    stop=(kc == KC - 1),
                )
            nc.vector.tensor_scalar(
                out=v_sb[:, tt, :, 0:64],
                in0=psv.rearrange("p (h d) -> p h d", h=HPC),
                scalar1=r_tok[:, tt : tt + 1],
                scalar2=None,
                op0=mult,
            )

        def emit_outproj_do(qtp, do):
            pool = state["sc_pool"]
            tg = "proj" if pool is psKQ else "sc"
            po = pool.tile([128, 512], f32, tag=tg, name=f"o_{qtp}_{do}")
            for fc in range(2):
                nc.tensor.matmul(
                    po,
                    lhsT=wo_sb[:, fc, do * 128 : (do + 1) * 128],
                    rhs=av_all[:, fc, qtp * 512 : (qtp + 1) * 512],
                    start=(fc == 0),
                    stop=(fc == 1),
                )
            ob = work.tile([128, 512], bf16, tag="ob")
            if qtp == 3:
                nc.scalar.copy(out=ob, in_=po)  # Act is idle in the tail
            else:
                nc.vector.tensor_copy(out=ob, in_=po)
            nc.sync.dma_start(
                out.rearrange("(do p) t -> p do t", p=128)[
                    :, do, qtp * 512 : (qtp + 1) * 512
                ],
                ob,
            )

        def run_quarter(qt, av_pool):
            q0 = qt * 512
            pool = state["sc_pool"]
            tg = "proj" if pool is psKQ else "sc"
            split = state["split_sc"]
            for pi in range(2):
                # full-bank tiles: matmul start=True zeroes the whole 2KB
                # bank, so only the FIRST write into each bank uses it
                av4 = [
                    av_pool.tile(
                        [128, 4, 128], f32,
                        tag="proj" if av_pool is psKQ else "av4",
                        name=f"av_{qt}_{pi}_{x}",
                    )
                    for x in range(2)
                ]

                def emit_av(kb, ex):
                    for qbl in range(4):
                        qb = 4 * qt + qbl
                        if kb > qb:
                            continue
                        for x in range(2):
                            nc.tensor.matmul(
                                av4[x][:, qbl, 0:65],
                                lhsT=ex[:, x * 512 + qbl * 128 : x * 512 + (qbl + 1) * 128],
                                rhs=v_sb[:, kb, 2 * pi + x, :],
                                start=(kb == 0 and qbl == 0),
                                stop=(kb == qb),
                                skip_group_check=True,
                            )

                nkb = 4 * qt + 4
                pend = None
                for kb in range(nkb + 1):
                    cur = None
                    if kb < nkb:
                        if pi == 0 and qt == 0:
                            v_proj(kb)
                        c0 = max(0, kb * 128 - q0)
                        if split:
                            scs = [
                                pool.tile([128, 512], f32, tag=tg,
                                          name=f"sc_{qt}_{pi}_{kb}_{x}")
                                for x in range(2)
                            ]
                        else:
                            scp = pool.tile([128, 1024], f32, tag=tg,
                                            name=f"sc_{qt}_{pi}_{kb}")
                            scs = [scp[:, 0:512], scp[:, 512:1024]]
                        for x in range(2):
                            rX = slice(x * 64, x * 64 + 64)
                            nc.tensor.matmul(
                                scs[x][:, c0:512],
                                lhsT=qk_sb[rX, 2 + pi, kb * 128 : (kb + 1) * 128],
                                rhs=qk_sb[rX, pi, q0 + c0 : q0 + 512],
                                start=True,
                                stop=True,
                            )
                        if pi == 0 and qt > 0 and kb < 4:
                            v_proj(4 * qt + kb)
                        # rope the NEXT quarter's token slice, one projection
                        # per kb iteration of pair 1
                        if pi == 1 and qt < 3 and kb < 4:
                            rope_tt((2, 0, 3, 1)[kb], qt + 1, pool)
                        if pi == 0 and state["pending_oq"] is not None and kb >= 2:
                            qtp, nd = state["pending_oq"]
                            todo = 8 - nd
                            left = nkb - kb
                            n_emit = -(-todo // max(left, 1))
                            for _ in range(min(n_emit, todo)):
                                emit_outproj_do(qtp, nd)
                                nd += 1
                            state["pending_oq"] = (qtp, nd) if nd < 8 else None
                        ex = expp.tile([128, 1024], bf16, tag="exp")
                        if split or c0 > 0:
                            for x in range(2):
                                nc.scalar.activation(
                                    ex[:, x * 512 + c0 : x * 512 + 512],
                                    scs[x][:, c0:512],
                                    Exp,
                                    scale=r_tok[:, kb : kb + 1],
                                )
                        else:
                            nc.scalar.activation(
                                ex, scp, Exp, scale=r_tok[:, kb : kb + 1]
                            )
                        if kb >= 4 * qt:  # diagonal block: causal mask
                            for x in range(2):
                                nc.gpsimd.tensor_tensor(
                                    ex[:, x * 512 + c0 : x * 512 + c0 + 128],
                                    ex[:, x * 512 + c0 : x * 512 + c0 + 128],
                                    mask_sb,
                                    mult,
                                )
                        cur = (kb, ex)
                    if pend is not None:
                        emit_av(*pend)
                    pend = cur
                # normalize (rows 0..63 / row 64) into token-major av_tok
                rec4s = []
                for x in range(2):
                    rec4 = recp.tile([128, 4], f32, tag="rec")
                    nc.vector.reciprocal(rec4, av4[x][:, :, 64:65])
                    rec4s.append(rec4)
                for qbl in range(4):
                    for x in range(2):
                        h = 2 * pi + x
                        nc.vector.tensor_scalar(
                            out=av_tok[:, 4 * qt + qbl, h * 64 : (h + 1) * 64],
                            in0=av4[x][:, qbl, 0:64],
                            scalar1=rec4s[x][:, qbl : qbl + 1],
                            scalar2=None,
                            op0=mult,
                        )
            # back to feature-major via PE transposes (53ns each)
            avT = pool.tile([128, 8, 128], bf16, tag=tg, name=f"avt_{qt}")
            for j, tt in enumerate(range(4 * qt, 4 * qt + 4)):
                for fc in range(2):
                    nc.tensor.transpose(
                        avT[:, fc * 4 + j, :],
                        av_tok[:, tt, fc * 128 : (fc + 1) * 128],
                        id_sb,
                    )
            for fc in range(2):
                nc.vector.tensor_copy(
                    out=av_all[:, fc, q0 : q0 + 512],
                    in_=avT[:, fc * 4 : fc * 4 + 4, :],
                )
            if qt < 3:
                # out-projection deferred into the next quarter's kb loop
                state["pending_oq"] = (qt, 0)
            else:
                for do in range(8):
                    emit_outproj_do(3, do)

        if QT0_IN_RING:
            # quarter 0 inside the psKQ ring, overlapping the QKV tail
            run_quarter(0, psKQ)
        ctxA.close()
        with (
            tc.tile_pool(name="psSC", bufs=3, space="PSUM") as psSC,
            tc.tile_pool(name="psAV", bufs=2, space="PSUM") as psAV,
        ):
            state["sc_pool"] = psSC
            state["split_sc"] = False
            for qt in range(0 if not QT0_IN_RING else 1, 4):
                run_quarter(qt, psAV)
            if tap == "rtok":
                nc.sync.dma_start(dbg[:, :], r_tok)
            elif tap == "qk":
                nc.sync.dma_start(dbg[:, :, :], qk_sb)
            elif tap == "v":
                nc.sync.dma_start(dbg[:, :, :, :], v_sb)
            elif tap == "avtok":
                nc.sync.dma_start(dbg[:, :, :], av_tok)
            elif tap == "avall":
                nc.sync.dma_start(dbg[:, :, :], av_all)
    nc.compile()
    return nc


def _host_inputs(x, norm_w, w_qkv, w_o, sin, cos):
    """Build the 8 per-core input maps (all bf16)."""
    n = T
    w_eff = np.asarray(w_qkv, np.float64) * np.asarray(norm_w, np.float64)[:, None]
    sin_n = np.asarray(sin, np.float32)[:n]  # [T, 64]
    cos_n = np.asarray(cos, np.float32)[:n]
    sign = np.concatenate([-np.ones(32, np.float32), np.ones(32, np.float32)])
    cos_tile = np.tile(cos_n.T, (2, 1))  # [128, T]
    sin_tile = np.tile((sin_n * sign[None, :]).T, (2, 1))  # [128, T]
    perm = np.zeros((128, 128), np.float32)
    for m in range(128):
        d = m % 64
        k = m + 32 if d < 32 else m - 32
        perm[k, m] = 1.0
    ident_np = np.eye(128, dtype=np.float32)
    ql = np.arange(128)[None, :]
    key = np.arange(128)[:, None]
    masks = (ql >= key).astype(np.float32)

    in_maps = []
    for c in range(8):
        b, g = c // 4, c % 4
        fs = slice(g * F, (g + 1) * F)
        in_maps.append(
            {
                "xT": np.ascontiguousarray(np.asarray(x, np.float32)[b].T).astype(BF16),
                "wq": (w_eff[:, 0:DIM][:, fs] * (DIM_HEAD ** -0.5)).astype(BF16),
                "wk": w_eff[:, DIM : 2 * DIM][:, fs].astype(BF16),
                "wv": w_eff[:, 2 * DIM : 3 * DIM][:, fs].astype(BF16),
                "wo": np.asarray(w_o, np.float32)[fs, :].astype(BF16),
                "cosT": cos_tile.astype(BF16),
                "sinT": sin_tile.astype(BF16),
                "perm": perm.astype(BF16),
                "masks": masks.astype(BF16),
                "ident": ident_np.astype(BF16),
            }
        )
    return in_maps


def kernel(x, norm_w, w_qkv, w_o, b_o, sin, cos):
    from concourse.bass_utils import run_bass_kernel_spmd

    if "nc" not in _NC_CACHE:
        _NC_CACHE["nc"] = _build_nc()
    nc = _NC_CACHE["nc"]
    in_maps = _host_inputs(x, norm_w, w_qkv, w_o, sin, cos)
    trace = bool(int(os.environ.get("KERNEL_TRACE", "0")))
    res = run_bass_kernel_spmd(nc, in_maps, core_ids=list(range(8)), trace=trace)
    if trace and res.exec_time_ns is not None:
        print(f"HW exec time: {res.exec_time_ns} ns")
    outs = [r["out"].astype(np.float32) for r in res.results]  # [1024, T] fm
    b_o = np.asarray(b_o, np.float32)
    full = np.empty((B, T, DIM), np.float32)
    for b in range(B):
        acc = outs[b * 4] + outs[b * 4 + 1] + outs[b * 4 + 2] + outs[b * 4 + 3]
        full[b] = acc.T + b_o[None, :]
    return full


# revision 81
# speedup vs baseline: 1.1111x; 1.0007x over previous
"""Trainium2 8-core kernel for RMSNorm -> QKV -> RoPE -> causal SDPA -> out-proj.

Sharding: core c = b*4 + g handles batch b (of 2) and heads 4g..4g+3 (of 16).
Each core computes a partial out-projection [dim, tokens]; the host sums the
4 head-group partials per batch (the tensor-parallel "unshard") and adds b_o.

Cost-model-driven layout (TimelineSim charges matmuls by OUTPUT FREE SIZE
only — contraction depth and output partitions are free):
  - scores per (head, kb): [key 128, q free] trimmed to the causal triangle.
  - AV runs TRANSPOSED: out [q 128, d 65] so each accumulation step costs 65
    rows instead of ~512; the ones column gives the softmax denominator.
    (matmul start=True zeroes the whole 2KB PSUM bank, so av4 tiles are
    bank-sized and only the first write uses start=True.)
  - The normalized token-major AV result returns to feature-major layout via
    PE transposes (53ns per 128x128 block) + one DVE copy per fc half.
  - exp for a head PAIR is fused into one Activation instruction (the two
    heads' score tiles sit in adjacent PSUM banks); Sqrt/Exp act tables are
    preloaded at t=0 with dummy activations.
  - r = rsqrt(mean x^2) rides into Q via r-scaled RoPE tables, into scores
    via the per-key `scale` operand of exp, and into V via a per-partition
    tensor_scalar during the PSUM->SBUF copy. r_tok (token-major r) comes
    from 16 free PE transposes of the r row.
  - Latency hiding: K/Q-fc0 project together chunk-paced off the DMA stream
    (PE idle gaps halve PE speed until 3us of continuous execution); the
    ss/r-chain, K/Q-fc1 and the tt0 RoPE interleave in wave 2; V projection,
    next-quarter RoPE, and the previous quarter's out-projection are all
    spread through the attention kb loops so the scalar engine (exp) stays
    fed; the last quarter's out-proj copies ride the then-idle Act engine.
"""

import os

import numpy as np
import ml_dtypes

BF16 = ml_dtypes.bfloat16

DIM = 1024
HEADS = 16
DIM_HEAD = 64
T = 2048  # tokens per batch
B = 2
HPC = 4  # heads per core
F = HPC * DIM_HEAD  # 256 per-core head width
KC = DIM // 128  # 8 contraction chunks
KORD = [2, 3, 4, 5, 6, 7, 0, 1]  # kc order: first matmul waits for chunk 2
TAIL_FINE = int(os.environ.get("KTAIL", "0"))
QT0_IN_RING = int(os.environ.get("KQT0", "0"))

_NC_CACHE = {}


def _build_nc():
    import concourse.bacc as bacc
    import concourse.mybir as mybir
    import concourse.tile as tile
    from contextlib import ExitStack

    f32 = mybir.dt.float32
    bf16 = mybir.dt.bfloat16
    nc = bacc.Bacc()

    xT = nc.declare_dram_parameter("xT", [DIM, T], bf16, isOutput=False)
    wq = nc.declare_dram_parameter("wq", [DIM, F], bf16, isOutput=False)
    wk = nc.declare_dram_parameter("wk", [DIM, F], bf16, isOutput=False)
    wv = nc.declare_dram_parameter("wv", [DIM, F], bf16, isOutput=False)
    wo = nc.declare_dram_parameter("wo", [F, DIM], bf16, isOutput=False)
    cosT = nc.declare_dram_parameter("cosT", [128, T], bf16, isOutput=False)
    sinT = nc.declare_dram_parameter("sinT", [128, T], bf16, isOutput=False)
    perm = nc.declare_dram_parameter("perm", [128, 128], bf16, isOutput=False)
    masks = nc.declare_dram_parameter("masks", [128, 128], bf16, isOutput=False)
    ident = nc.declare_dram_parameter("ident", [128, 128], bf16, isOutput=False)
    out = nc.declare_dram_parameter("out", [DIM, T], bf16, isOutput=True)
    tap = os.environ.get("KTAP", "")
    dbg = None
    if tap:
        _tap_shapes = {
            "rtok": ([128, 16], f32),
            "qk": ([128, 4, T], bf16),
            "v": ([128, 16, HPC, 65], bf16),
            "avtok": ([128, 16, F], bf16),
            "avall": ([128, 2, T], bf16),
        }
        shp, dt = _tap_shapes[tap]
        dbg = nc.declare_dram_parameter("dbg", shp, dt, isOutput=True)

    Exp = mybir.ActivationFunctionType.Exp
    Sqrt = mybir.ActivationFunctionType.Sqrt
    mult = mybir.AluOpType.mult
    add = mybir.AluOpType.add

    with ExitStack() as ctx:
        tc = ctx.enter_context(tile.TileContext(nc))
        consts = ctx.enter_context(tc.tile_pool(name="consts", bufs=1))
        persist = ctx.enter_context(tc.tile_pool(name="persist", bufs=1))
        work = ctx.enter_context(tc.tile_pool(name="work", bufs=4))
        vecs = ctx.enter_context(tc.tile_pool(name="vecs", bufs=1))

        # ---- constants / inputs ----
        wk_sb = consts.tile([128, KC, F], bf16, tag="wk")
        wq_sb = consts.tile([128, KC, F], bf16, tag="wq")
        wv_sb = consts.tile([128, KC, F], bf16, tag="wv")
        wo_sb = consts.tile([128, 2, DIM], bf16, tag="wo")
        cos_sb = consts.tile([128, T], bf16, tag="cos")
        sin_sb = consts.tile([128, T], bf16, tag="sin")
        perm_sb = consts.tile([128, 128], bf16, tag="perm")
        mask_sb = consts.tile([128, 128], bf16, tag="mask")
        id_sb = consts.tile([128, 128], bf16, tag="ident")
        ones_col = consts.tile([128, 1], bf16, tag="onesc")
        one_f32 = consts.tile([1, 1], f32, tag="onef")
        xT_sb = persist.tile([128, KC, T], bf16, tag="xT")
        xT_r = xT.rearrange("(kc p) t -> p kc t", p=128)
        # wk first (first PE consumer), then xT chunks in consumption order
        # with the other weights slotted behind the early chunks
        nc.sync.dma_start(wk_sb, wk.rearrange("(kc p) f -> p kc f", p=128))
        nc.sync.dma_start(wq_sb, wq.rearrange("(kc p) f -> p kc f", p=128))
        for kc in KORD[:4]:
            nc.sync.dma_start(xT_sb[:, kc], xT_r[:, kc])
        nc.sync.dma_start(perm_sb, perm[:, :])
        nc.sync.dma_start(cos_sb, cosT[:, :])
        nc.sync.dma_start(sin_sb, sinT[:, :])
        for kc in KORD[4:]:
            nc.sync.dma_start(xT_sb[:, kc], xT_r[:, kc])
        nc.sync.dma_start(wv_sb, wv.rearrange("(kc p) f -> p kc f", p=128))
        nc.sync.dma_start(mask_sb, masks[:, :])
        nc.sync.dma_start(id_sb, ident[:, :])
        nc.sync.dma_start(wo_sb, wo.rearrange("(fc p) d -> p fc d", p=128))
        nc.vector.memset(ones_col, 1.0)
        nc.vector.memset(one_f32, 1.0)

        # persistent activations
        qk_sb = persist.tile([128, 4, T], bf16, tag="qk")  # 0,1=q fc0/1; 2,3=k
        v_sb = persist.tile([128, 16, HPC, 65], bf16, tag="v")
        av_tok = persist.tile([128, 16, F], bf16, tag="avtok")
        av_all = persist.tile([128, 2, T], bf16, tag="av")
        r_sb = vecs.tile([1, T], f32, tag="r")
        r_tok = vecs.tile([128, 16], f32, tag="rtok")
        r_bc = persist.tile([128, T], f32, tag="rbc")
        cosr_sb = persist.tile([128, T], bf16, tag="cosr")
        sinr_sb = persist.tile([128, T], bf16, tag="sinr")
        qraw_sb = persist.tile([128, 2, 4, 512], bf16, tag="qraw")
        kraw_sb = persist.tile([128, 2, 4, 512], bf16, tag="kraw")
        nc.vector.memset(v_sb[:, :, :, 64:65], 1.0)
        # preload the Sqrt/Exp activation tables while DMAs stream in
        dum = vecs.tile([1, 1], f32, tag="dum")
        nc.scalar.activation(dum, one_f32, Sqrt)
        nc.scalar.activation(dum, dum, Exp)

        expp = ctx.enter_context(tc.tile_pool(name="expp", bufs=8))
        recp = ctx.enter_context(tc.tile_pool(name="recp", bufs=4))

        ctxA = ExitStack()
        psKQ = ctxA.enter_context(tc.tile_pool(name="psKQ", bufs=8, space="PSUM"))
        sbA = ctxA.enter_context(tc.tile_pool(name="sbA", bufs=1))
        xsq_sb = sbA.tile([128, KC, T], bf16, tag="xsq")

        # x^2 per chunk (DVE, chases the xT DMAs)
        for kc in KORD:
            nc.vector.tensor_mul(xsq_sb[:, kc], xT_sb[:, kc], xT_sb[:, kc])

        def rope_tt(fidx, tt, pool):
            """RoPE one 512-token slice of Q/K from the raw SBUF copy:
            rotate-half perm matmul + two multiplies + add into qk_sb.
            Q (fidx 0,1) uses the r-scaled tables so r_q rides in free."""
            ts = slice(tt * 512, (tt + 1) * 512)
            is_q = fidx < 2
            raw = (qraw_sb if is_q else kraw_sb)[:, fidx % 2, tt]
            cc = cosr_sb if is_q else cos_sb
            ssb = sinr_sb if is_q else sin_sb
            pp = pool.tile([128, 512], f32, tag="sc" if pool is not psKQ else "proj",
                           name=f"pp_{fidx}_{tt}")
            nc.tensor.matmul(pp, lhsT=perm_sb, rhs=raw, start=True, stop=True)
            t1 = work.tile([128, 512], bf16, tag="t1")
            nc.vector.tensor_tensor(t1, pp, ssb[:, ts], mult)
            t2 = work.tile([128, 512], bf16, tag="t2")
            nc.vector.tensor_tensor(t2, raw, cc[:, ts], mult)
            nc.vector.tensor_tensor(qk_sb[:, fidx, ts], t2, t1, add)

        # ---- wave 1: K-fc0 + Q-fc0 projections, chunk-paced off DMA ----
        psW = {}
        for nm in ("k0", "q0"):
            for tt in range(4):
                psW[(nm, tt)] = psKQ.tile(
                    [128, 512], f32, tag="proj", name=f"{nm}_{tt}"
                )
        for kc in KORD:
            for tt in range(4):
                ts = slice(tt * 512, (tt + 1) * 512)
                nc.tensor.matmul(
                    psW[("k0", tt)],
                    lhsT=wk_sb[:, kc, 0:128],
                    rhs=xT_sb[:, kc, ts],
                    start=(kc == KORD[0]),
                    stop=(kc == KORD[-1]),
                )
                nc.tensor.matmul(
                    psW[("q0", tt)],
                    lhsT=wq_sb[:, kc, 0:128],
                    rhs=xT_sb[:, kc, ts],
                    start=(kc == KORD[0]),
                    stop=(kc == KORD[-1]),
                )
        # free the k0 slots first (Act; DVE is still finishing x^2);
        # q0 copies are deferred into wave 2 so the r-chain starts sooner
        for tt in range(4):
            nc.scalar.copy(out=kraw_sb[:, 0, tt], in_=psW[("k0", tt)])

        # ---- wave 2: ss/r-chain + K-fc1 + Q-fc1 interleaved ----
        ss_sb = sbA.tile([1, T], f32, tag="ss")

        def proj_fc1(which, tt):
            w = wk_sb if which == "k" else wq_sb
            psq = psKQ.tile([128, 512], f32, tag="proj", name=f"{which}1_{tt}")
            for kc in range(KC):
                nc.tensor.matmul(
                    psq,
                    lhsT=w[:, kc, 128:256],
                    rhs=xT_sb[:, kc, tt * 512 : (tt + 1) * 512],
                    start=(kc == 0),
                    stop=(kc == KC - 1),
                )
            if which == "k":
                nc.vector.tensor_copy(out=kraw_sb[:, 1, tt], in_=psq)
            else:
                nc.scalar.copy(out=qraw_sb[:, 1, tt], in_=psq)

        def ss_slice(s):
            ts = slice(s * 512, (s + 1) * 512)
            ss_ps = psKQ.tile([1, 512], f32, tag="proj", name=f"ss_{s}")
            for kc in range(KC):
                nc.tensor.matmul(
                    ss_ps,
                    lhsT=ones_col,
                    rhs=xsq_sb[:, kc, s * 512 : (s + 1) * 512],
                    start=(kc == 0),
                    stop=(kc == KC - 1),
                )
            nc.scalar.activation(
                ss_sb[:, ts], ss_ps, Sqrt, scale=1.0 / DIM
            )
            nc.vector.reciprocal(r_sb[:, ts], ss_sb[:, ts])
            nc.gpsimd.partition_broadcast(r_bc[:, ts], r_sb[:, ts])
            nc.gpsimd.tensor_tensor(cosr_sb[:, ts], cos_sb[:, ts], r_bc[:, ts], mult)
            nc.gpsimd.tensor_tensor(sinr_sb[:, ts], sin_sb[:, ts], r_bc[:, ts], mult)

        proj_fc1("q", 0)
        ss_slice(0)
        for tt in (0, 1):
            nc.scalar.copy(out=qraw_sb[:, 0, tt], in_=psW[("q0", tt)])
        rope_tt(2, 0, psKQ)
        proj_fc1("k", 0)
        ss_slice(1)
        rope_tt(0, 0, psKQ)
        for tt in (2, 3):
            nc.scalar.copy(out=qraw_sb[:, 0, tt], in_=psW[("q0", tt)])
        proj_fc1("q", 1)
        rope_tt(3, 0, psKQ)
        proj_fc1("k", 1)
        rope_tt(1, 0, psKQ)
        ss_slice(2)
        proj_fc1("q", 2)
        proj_fc1("k", 2)
        ss_slice(3)
        # r_tok via PE transposes of the r row
        rtok_ps = psKQ.tile([128, 16], f32, tag="proj", name="rtokps")
        for i in range(16):
            nc.tensor.transpose(
                rtok_ps[:, i : i + 1], r_sb[0:1, i * 128 : (i + 1) * 128],
                one_f32,
            )
        nc.vector.tensor_copy(out=r_tok, in_=rtok_ps)
        proj_fc1("q", 3)
        proj_fc1("k", 3)

        # ---- attention: scores [k,q] -> paired exp -> transposed AV ----
        # Quarter 0 runs INSIDE the psKQ ring (overlapping the QKV tail);
        # quarters 1-3 use dedicated pools: sc ring (3x2 banks, also V/pp/
        # outproj/avT) + av4 ring (2 banks).
        state = {"pending_oq": None, "sc_pool": psKQ, "split_sc": True}

        def v_proj(tt):
            pool = state["sc_pool"]
            tg = "proj" if pool is psKQ else "sc"
            psv = pool.tile([128, 256], f32, tag=tg, name=f"v_{tt}")
            for kc in range(KC):
                nc.tensor.matmul(
                    psv,
                    lhsT=xT_sb[:, kc, tt * 128 : (tt + 1) * 128],
                    rhs=wv_sb[:, kc, :],
                    start=(kc == 0),
                    stop=(kc == KC - 1),
                )
            nc.vector.tensor_scalar(
                out=v_sb[:, tt, :, 0:64],
                in0=psv.rearrange("p (h d) -> p h d", h=HPC),
                scalar1=r_tok[:, tt : tt + 1],
                scalar2=None,
                op0=mult,
            )

        def emit_outproj_do(qtp, do):
            pool = state["sc_pool"]
            tg = "proj" if pool is psKQ else "sc"
            po = pool.tile([128, 512], f32, tag=tg, name=f"o_{qtp}_{do}")
            for fc in range(2):
                nc.tensor.matmul(
                    po,
                    lhsT=wo_sb[:, fc, do * 128 : (do + 1) * 128],
                    rhs=av_all[:, fc, qtp * 512 : (qtp + 1) * 512],
                    start=(fc == 0),
                    stop=(fc == 1),
                )
            ob = work.tile([128, 512], bf16, tag="ob")
            if qtp == 3:
                nc.scalar.copy(out=ob, in_=po)  # Act is idle in the tail
            else:
                nc.vector.tensor_copy(out=ob, in_=po)
            nc.sync.dma_start(
                out.rearrange("(do p) t -> p do t", p=128)[
                    :, do, qtp * 512 : (qtp + 1) * 512
                ],
                ob,
            )

        def run_quarter(qt, av_pool):
            q0 = qt * 512
            pool = state["sc_pool"]
            tg = "proj" if pool is psKQ else "sc"
            split = state["split_sc"]
            for pi in range(2):
                # full-bank tiles: matmul start=True zeroes the whole 2KB
                # bank, so only the FIRST write into each bank uses it
                av4 = [
                    av_pool.tile(
                        [128, 4, 128], f32,
                        tag="proj" if av_pool is psKQ else "av4",
                        name=f"av_{qt}_{pi}_{x}",
                    )
                    for x in range(2)
                ]

                def emit_av(kb, ex):
                    for qbl in range(4):
                        qb = 4 * qt + qbl
                        if kb > qb:
                            continue
                        for x in range(2):
                            nc.tensor.matmul(
                                av4[x][:, qbl, 0:65],
                                lhsT=ex[:, x * 512 + qbl * 128 : x * 512 + (qbl + 1) * 128],
                                rhs=v_sb[:, kb, 2 * pi + x, :],
                                start=(kb == 0 and qbl == 0),
                                stop=(kb == qb),
                                skip_group_check=True,
                            )

                nkb = 4 * qt + 4
                pend = None
                for kb in range(nkb + 1):
                    cur = None
                    if kb < nkb:
                        if pi == 0 and qt == 0:
                            v_proj(kb)
                        c0 = max(0, kb * 128 - q0)
                        if split:
                            scs = [
                                pool.tile([128, 512], f32, tag=tg,
                                          name=f"sc_{qt}_{pi}_{kb}_{x}")
                                for x in range(2)
                            ]
                        else:
                            scp = pool.tile([128, 1024], f32, tag=tg,
                                            name=f"sc_{qt}_{pi}_{kb}")
                            scs = [scp[:, 0:512], scp[:, 512:1024]]
                        for x in range(2):
                            rX = slice(x * 64, x * 64 + 64)
                            nc.tensor.matmul(
                                scs[x][:, c0:512],
                                lhsT=qk_sb[rX, 2 + pi, kb * 128 : (kb + 1) * 128],
                                rhs=qk_sb[rX, pi, q0 + c0 : q0 + 512],
                                start=True,
                                stop=True,
                            )
                        if pi == 0 and qt > 0 and kb < 4:
                            v_proj(4 * qt + kb)
                        # rope the NEXT quarter's token slice, one projection
                        # per kb iteration of pair 1
                        if pi == 1 and qt < 3 and kb < 4:
                            rope_tt((2, 0, 3, 1)[kb], qt + 1, pool)
                        if pi == 0 and state["pending_oq"] is not None and kb >= 2:
                            qtp, nd = state["pending_oq"]
                            todo = 8 - nd
                            left = nkb - kb
                            n_emit = -(-todo // max(left, 1))
                            for _ in range(min(n_emit, todo)):
                                emit_outproj_do(qtp, nd)
                                nd += 1
                            state["pending_oq"] = (qtp, nd) if nd < 8 else None
                        ex = expp.tile([128, 1024], bf16, tag="exp")
                        if split or c0 > 0:
                            for x in range(2):
                                nc.scalar.activation(
                                    ex[:, x * 512 + c0 : x * 512 + 512],
                                    scs[x][:, c0:512],
                                    Exp,
                                    scale=r_tok[:, kb : kb + 1],
                                )
                        else:
                            nc.scalar.activation(
                                ex, scp, Exp, scale=r_tok[:, kb : kb + 1]
                            )
                        if kb >= 4 * qt:  # diagonal block: causal mask
                            for x in range(2):
                                nc.gpsimd.tensor_tensor(
                                    ex[:, x * 512 + c0 : x * 512 + c0 + 128],
                                    ex[:, x * 512 + c0 : x * 512 + c0 + 128],
                                    mask_sb,
                                    mult,
                                )
                        cur = (kb, ex)
                    if pend is not None:
                        emit_av(*pend)
                    pend = cur
                # normalize (rows 0..63 / row 64) into token-major av_tok
                rec4s = []
                for x in range(2):
                    rec4 = recp.tile([128, 4], f32, tag="rec")
                    nc.vector.reciprocal(rec4, av4[x][:, :, 64:65])
                    rec4s.append(rec4)
                for qbl in range(4):
                    for x in range(2):
                        h = 2 * pi + x
                        nc.vector.tensor_scalar(
                            out=av_tok[:, 4 * qt + qbl, h * 64 : (h + 1) * 64],
                            in0=av4[x][:, qbl, 0:64],
                            scalar1=rec4s[x][:, qbl : qbl + 1],
                            scalar2=None,
                            op0=mult,
                        )
            # back to feature-major via PE transposes (53ns each)
            avT = pool.tile([128, 8, 128], bf16, tag=tg, name=f"avt_{qt}")
            for j, tt in enumerate(range(4 * qt, 4 * qt + 4)):
                for fc in range(2):
                    nc.tensor.transpose(
                        avT[:, fc * 4 + j, :],
                        av_tok[:, tt, fc * 128 : (fc + 1) * 128],
                        id_sb,
                    )
            for fc in range(2):
                nc.vector.tensor_copy(
                    out=av_all[:, fc, q0 : q0 + 512],
                    in_=avT[:, fc * 4 : fc * 4 + 4, :],
                )
            if qt < 3:
                # out-projection deferred into the next quarter's kb loop
                state["pending_oq"] = (qt, 0)
            else:
                for do in range(8):
                    emit_outproj_do(3, do)

        if QT0_IN_RING:
            # quarter 0 inside the psKQ ring, overlapping the QKV tail
            run_quarter(0, psKQ)
        ctxA.close()
        with (
            tc.tile_pool(name="psSC", bufs=3, space="PSUM") as psSC,
            tc.tile_pool(name="psAV", bufs=2, space="PSUM") as psAV,
        ):
            state["sc_pool"] = psSC
            state["split_sc"] = False
            for qt in range(0 if not QT0_IN_RING else 1, 4):
                run_quarter(qt, psAV)
            if tap == "rtok":
                nc.sync.dma_start(dbg[:, :], r_tok)
            elif tap == "qk":
                nc.sync.dma_start(dbg[:, :, :], qk_sb)
            elif tap == "v":
                nc.sync.dma_start(dbg[:, :, :, :], v_sb)
            elif tap == "avtok":
                nc.sync.dma_start(dbg[:, :, :], av_tok)
            elif tap == "avall":
                nc.sync.dma_start(dbg[:, :, :], av_all)
    nc.compile()
    return nc


def _host_inputs(x, norm_w, w_qkv, w_o, sin, cos):
    """Build the 8 per-core input maps (all bf16)."""
    n = T
    w_eff = np.asarray(w_qkv, np.float64) * np.asarray(norm_w, np.float64)[:, None]
    sin_n = np.asarray(sin, np.float32)[:n]  # [T, 64]
    cos_n = np.asarray(cos, np.float32)[:n]
    sign = np.concatenate([-np.ones(32, np.float32), np.ones(32, np.float32)])
    cos_tile = np.tile(cos_n.T, (2, 1))  # [128, T]
    sin_tile = np.tile((sin_n * sign[None, :]).T, (2, 1))  # [128, T]
    perm = np.zeros((128, 128), np.float32)
    for m in range(128):
        d = m % 64
        k = m + 32 if d < 32 else m - 32
        perm[k, m] = 1.0
    ident_np = np.eye(128, dtype=np.float32)
    ql = np.arange(128)[None, :]
    key = np.arange(128)[:, None]
    masks = (ql >= key).astype(np.float32)

    in_maps = []
    for c in range(8):
        b, g = c // 4, c % 4
        fs = slice(g * F, (g + 1) * F)
        xT_np = np.ascontiguousarray(np.asarray(x, np.float32)[b].T).astype(BF16)
        in_maps.append(
            {
                "xT": xT_np,
                "wq": (w_eff[:, 0:DIM][:, fs] * (DIM_HEAD ** -0.5)).astype(BF16),
                "wk": w_eff[:, DIM : 2 * DIM][:, fs].astype(BF16),
                "wv": w_eff[:, 2 * DIM : 3 * DIM][:, fs].astype(BF16),
                "wo": np.asarray(w_o, np.float32)[fs, :].astype(BF16),
                "cosT": cos_tile.astype(BF16),
                "sinT": sin_tile.astype(BF16),
                "perm": perm.astype(BF16),
                "masks": masks.astype(BF16),
                "ident": ident_np.astype(BF16),
            }
        )
    return in_maps


def kernel(x, norm_w, w_qkv, w_o, b_o, sin, cos):
    from concourse.bass_utils import run_bass_kernel_spmd

    if "nc" not in _NC_CACHE:
        _NC_CACHE["nc"] = _build_nc()
    nc = _NC_CACHE["nc"]
    in_maps = _host_inputs(x, norm_w, w_qkv, w_o, sin, cos)
    trace = bool(int(os.environ.get("KERNEL_TRACE", "0")))
    res = run_bass_kernel_spmd(nc, in_maps, core_ids=list(range(8)), trace=trace)
    if trace and res.exec_time_ns is not None:
        print(f"HW exec time: {res.exec_time_ns} ns")
    outs = [r["out"].astype(np.float32) for r in res.results]  # [1024, T] fm
    b_o = np.asarray(b_o, np.float32)
    full = np.empty((B, T, DIM), np.float32)
    for b in range(B):
        acc = outs[b * 4] + outs[b * 4 + 1] + outs[b * 4 + 2] + outs[b * 4 + 3]
        full[b] = acc.T + b_o[None, :]
    return full


# revision 85
# speedup vs baseline: 1.1125x; 1.0012x over previous
"""Trainium2 8-core kernel for RMSNorm -> QKV -> RoPE -> causal SDPA -> out-proj.

Sharding: core c = b*4 + g handles batch b (of 2) and heads 4g..4g+3 (of 16).
Each core computes a partial out-projection [dim, tokens]; the host sums the
4 head-group partials per batch (the tensor-parallel "unshard") and adds b_o.

Cost-model-driven layout (TimelineSim charges matmuls by OUTPUT FREE SIZE
only — contraction depth and output partitions are free):
  - scores per (head, kb): [key 128, q free] trimmed to the causal triangle.
  - AV runs TRANSPOSED: out [q 128, d 65] so each accumulation step costs 65
    rows instead of ~512; the ones column gives the softmax denominator.
    (matmul start=True zeroes the whole 2KB PSUM bank, so av4 tiles are
    bank-sized and only the first write uses start=True.)
  - The normalized token-major AV result returns to feature-major layout via
    PE transposes (53ns per 128x128 block) + one DVE copy per fc half.
  - exp for a head PAIR is fused into one Activation instruction (the two
    heads' score tiles sit in adjacent PSUM banks); Sqrt/Exp act tables are
    preloaded at t=0 with dummy activations.
  - r = rsqrt(mean x^2) rides into Q via r-scaled RoPE tables, into scores
    via the per-key `scale` operand of exp, and into V via a per-partition
    tensor_scalar during the PSUM->SBUF copy. r_tok (token-major r) comes
    from 16 free PE transposes of the r row.
  - Latency hiding: K/Q-fc0 project together chunk-paced off the DMA stream
    (PE idle gaps halve PE speed until 3us of continuous execution); the
    ss/r-chain, K/Q-fc1 and the tt0 RoPE interleave in wave 2; V projection,
    next-quarter RoPE, and the previous quarter's out-projection are all
    spread through the attention kb loops so the scalar engine (exp) stays
    fed; the last quarter's out-proj copies ride the then-idle Act engine.
"""

import os

import numpy as np
import ml_dtypes

BF16 = ml_dtypes.bfloat16

DIM = 1024
HEADS = 16
DIM_HEAD = 64
T = 2048  # tokens per batch
B = 2
HPC = 4  # heads per core
F = HPC * DIM_HEAD  # 256 per-core head width
KC = DIM // 128  # 8 contraction chunks
KORD = [2, 3, 4, 5, 6, 7, 0, 1]  # kc order: first matmul waits for chunk 2
TAIL_FINE = int(os.environ.get("KTAIL", "0"))
QT0_IN_RING = int(os.environ.get("KQT0", "0"))

_NC_CACHE = {}


def _build_nc():
    import concourse.bacc as bacc
    import concourse.mybir as mybir
    import concourse.tile as tile
    from contextlib import ExitStack

    f32 = mybir.dt.float32
    bf16 = mybir.dt.bfloat16
    nc = bacc.Bacc()

    xT = nc.declare_dram_parameter("xT", [DIM, T], bf16, isOutput=False)
    wq = nc.declare_dram_parameter("wq", [DIM, F], bf16, isOutput=False)
    wk = nc.declare_dram_parameter("wk", [DIM, F], bf16, isOutput=False)
    wv = nc.declare_dram_parameter("wv", [DIM, F], bf16, isOutput=False)
    wo = nc.declare_dram_parameter("wo", [F, DIM], bf16, isOutput=False)
    cosT = nc.declare_dram_parameter("cosT", [128, T], bf16, isOutput=False)
    sinT = nc.declare_dram_parameter("sinT", [128, T], bf16, isOutput=False)
    perm = nc.declare_dram_parameter("perm", [128, 128], bf16, isOutput=False)
    masks = nc.declare_dram_parameter("masks", [128, 128], bf16, isOutput=False)
    ident = nc.declare_dram_parameter("ident", [128, 128], bf16, isOutput=False)
    out = nc.declare_dram_parameter("out", [DIM, T], bf16, isOutput=True)
    tap = os.environ.get("KTAP", "")
    dbg = None
    if tap:
        _tap_shapes = {
            "rtok": ([128, 16], f32),
            "qk": ([128, 4, T], bf16),
            "v": ([128, 16, HPC, 65], bf16),
            "avtok": ([128, 16, F], bf16),
            "avall": ([128, 2, T], bf16),
        }
        shp, dt = _tap_shapes[tap]
        dbg = nc.declare_dram_parameter("dbg", shp, dt, isOutput=True)

    Exp = mybir.ActivationFunctionType.Exp
    Sqrt = mybir.ActivationFunctionType.Sqrt
    mult = mybir.AluOpType.mult
    add = mybir.AluOpType.add

    with ExitStack() as ctx:
        tc = ctx.enter_context(tile.TileContext(nc))
        consts = ctx.enter_context(tc.tile_pool(name="consts", bufs=1))
        persist = ctx.enter_context(tc.tile_pool(name="persist", bufs=1))
        work = ctx.enter_context(tc.tile_pool(name="work", bufs=4))
        vecs = ctx.enter_context(tc.tile_pool(name="vecs", bufs=1))

        # ---- constants / inputs ----
        wk_sb = consts.tile([128, KC, F], bf16, tag="wk")
        wq_sb = consts.tile([128, KC, F], bf16, tag="wq")
        wv_sb = consts.tile([128, KC, F], bf16, tag="wv")
        wo_sb = consts.tile([128, 2, DIM], bf16, tag="wo")
        cos_sb = consts.tile([128, T], bf16, tag="cos")
        sin_sb = consts.tile([128, T], bf16, tag="sin")
        perm_sb = consts.tile([128, 128], bf16, tag="perm")
        mask_sb = consts.tile([128, 128], bf16, tag="mask")
        id_sb = consts.tile([128, 128], bf16, tag="ident")
        ones_col = consts.tile([128, 1], bf16, tag="onesc")
        one_f32 = consts.tile([1, 1], f32, tag="onef")
        xT_sb = persist.tile([128, KC, T], bf16, tag="xT")
        xT_r = xT.rearrange("(kc p) t -> p kc t", p=128)
        # wk first (first PE consumer), then xT chunks in consumption order
        # with the other weights slotted behind the early chunks
        nc.sync.dma_start(wk_sb, wk.rearrange("(kc p) f -> p kc f", p=128))
        nc.sync.dma_start(wq_sb, wq.rearrange("(kc p) f -> p kc f", p=128))
        for kc in KORD[:4]:
            nc.sync.dma_start(xT_sb[:, kc], xT_r[:, kc])
        nc.sync.dma_start(perm_sb, perm[:, :])
        nc.sync.dma_start(cos_sb, cosT[:, :])
        nc.sync.dma_start(sin_sb, sinT[:, :])
        for kc in KORD[4:]:
            nc.sync.dma_start(xT_sb[:, kc], xT_r[:, kc])
        nc.sync.dma_start(wv_sb, wv.rearrange("(kc p) f -> p kc f", p=128))
        nc.sync.dma_start(mask_sb, masks[:, :])
        nc.sync.dma_start(id_sb, ident[:, :])
        nc.sync.dma_start(wo_sb, wo.rearrange("(fc p) d -> p fc d", p=128))
        nc.vector.memset(ones_col, 1.0)
        nc.vector.memset(one_f32, 1.0)

        # persistent activations
        qk_sb = persist.tile([128, 4, T], bf16, tag="qk")  # 0,1=q fc0/1; 2,3=k
        v_sb = persist.tile([128, 16, HPC, 65], bf16, tag="v")
        av_tok = persist.tile([128, 16, F], bf16, tag="avtok")
        av_all = persist.tile([128, 2, T], bf16, tag="av")
        r_sb = vecs.tile([1, T], f32, tag="r")
        r_tok = vecs.tile([128, 16], f32, tag="rtok")
        r_bc = persist.tile([128, T], f32, tag="rbc")
        cosr_sb = persist.tile([128, T], bf16, tag="cosr")
        sinr_sb = persist.tile([128, T], bf16, tag="sinr")
        qraw_sb = persist.tile([128, 2, 4, 512], bf16, tag="qraw")
        kraw_sb = persist.tile([128, 2, 4, 512], bf16, tag="kraw")
        nc.vector.memset(v_sb[:, :, :, 64:65], 1.0)
        # preload the Sqrt/Exp activation tables while DMAs stream in
        dum = vecs.tile([1, 1], f32, tag="dum")
        nc.scalar.activation(dum, one_f32, Sqrt)
        nc.scalar.activation(dum, dum, Exp)

        expp = ctx.enter_context(tc.tile_pool(name="expp", bufs=8))
        recp = ctx.enter_context(tc.tile_pool(name="recp", bufs=4))

        ctxA = ExitStack()
        psKQ = ctxA.enter_context(tc.tile_pool(name="psKQ", bufs=8, space="PSUM"))
        sbA = ctxA.enter_context(tc.tile_pool(name="sbA", bufs=1))
        xsq_sb = sbA.tile([128, KC, T], bf16, tag="xsq")

        # x^2 per chunk (DVE, chases the xT DMAs)
        for kc in KORD:
            nc.vector.tensor_mul(xsq_sb[:, kc], xT_sb[:, kc], xT_sb[:, kc])

        def rope_tt(fidx, tt, pool):
            """RoPE one 512-token slice of Q/K from the raw SBUF copy:
            rotate-half perm matmul + two multiplies + add into qk_sb.
            Q (fidx 0,1) uses the r-scaled tables so r_q rides in free."""
            ts = slice(tt * 512, (tt + 1) * 512)
            is_q = fidx < 2
            raw = (qraw_sb if is_q else kraw_sb)[:, fidx % 2, tt]
            cc = cosr_sb if is_q else cos_sb
            ssb = sinr_sb if is_q else sin_sb
            pp = pool.tile([128, 512], f32, tag="sc" if pool is not psKQ else "proj",
                           name=f"pp_{fidx}_{tt}")
            nc.tensor.matmul(pp, lhsT=perm_sb, rhs=raw, start=True, stop=True)
            t1 = work.tile([128, 512], bf16, tag="t1")
            nc.vector.tensor_tensor(t1, pp, ssb[:, ts], mult)
            t2 = work.tile([128, 512], bf16, tag="t2")
            nc.vector.tensor_tensor(t2, raw, cc[:, ts], mult)
            nc.vector.tensor_tensor(qk_sb[:, fidx, ts], t2, t1, add)

        # ---- wave 1: K-fc0 + Q-fc0 projections, chunk-paced off DMA ----
        psW = {}
        for nm in ("k0", "q0"):
            for tt in range(4):
                psW[(nm, tt)] = psKQ.tile(
                    [128, 512], f32, tag="proj", name=f"{nm}_{tt}"
                )
        for kc in KORD:
            for tt in range(4):
                ts = slice(tt * 512, (tt + 1) * 512)
                nc.tensor.matmul(
                    psW[("k0", tt)],
                    lhsT=wk_sb[:, kc, 0:128],
                    rhs=xT_sb[:, kc, ts],
                    start=(kc == KORD[0]),
                    stop=(kc == KORD[-1]),
                )
                nc.tensor.matmul(
                    psW[("q0", tt)],
                    lhsT=wq_sb[:, kc, 0:128],
                    rhs=xT_sb[:, kc, ts],
                    start=(kc == KORD[0]),
                    stop=(kc == KORD[-1]),
                )
        # free the k0 slots first (Act; DVE is still finishing x^2);
        # q0 copies are deferred into wave 2 so the r-chain starts sooner
        for tt in range(4):
            nc.scalar.copy(out=kraw_sb[:, 0, tt], in_=psW[("k0", tt)])

        # ---- wave 2: ss/r-chain + K-fc1 + Q-fc1 interleaved ----
        ss_sb = sbA.tile([1, T], f32, tag="ss")

        def proj_fc1(which, tt, pool=None):
            pool = pool if pool is not None else psKQ
            tg = "proj" if pool is psKQ else "sc"
            w = wk_sb if which == "k" else wq_sb
            psq = pool.tile([128, 512], f32, tag=tg, name=f"{which}1_{tt}")
            for kc in range(KC):
                nc.tensor.matmul(
                    psq,
                    lhsT=w[:, kc, 128:256],
                    rhs=xT_sb[:, kc, tt * 512 : (tt + 1) * 512],
                    start=(kc == 0),
                    stop=(kc == KC - 1),
                )
            if which == "k":
                nc.vector.tensor_copy(out=kraw_sb[:, 1, tt], in_=psq)
            else:
                nc.scalar.copy(out=qraw_sb[:, 1, tt], in_=psq)

        def ss_slice(s):
            ts = slice(s * 512, (s + 1) * 512)
            ss_ps = psKQ.tile([1, 512], f32, tag="proj", name=f"ss_{s}")
            for kc in range(KC):
                nc.tensor.matmul(
                    ss_ps,
                    lhsT=ones_col,
                    rhs=xsq_sb[:, kc, s * 512 : (s + 1) * 512],
                    start=(kc == 0),
                    stop=(kc == KC - 1),
                )
            nc.scalar.activation(
                ss_sb[:, ts], ss_ps, Sqrt, scale=1.0 / DIM
            )
            nc.vector.reciprocal(r_sb[:, ts], ss_sb[:, ts])
            nc.gpsimd.partition_broadcast(r_bc[:, ts], r_sb[:, ts])
            nc.gpsimd.tensor_tensor(cosr_sb[:, ts], cos_sb[:, ts], r_bc[:, ts], mult)
            nc.gpsimd.tensor_tensor(sinr_sb[:, ts], sin_sb[:, ts], r_bc[:, ts], mult)

        proj_fc1("q", 0)
        ss_slice(0)
        for tt in (0, 1):
            nc.scalar.copy(out=qraw_sb[:, 0, tt], in_=psW[("q0", tt)])
        rope_tt(2, 0, psKQ)
        proj_fc1("k", 0)
        ss_slice(1)
        rope_tt(0, 0, psKQ)
        for tt in (2, 3):
            nc.scalar.copy(out=qraw_sb[:, 0, tt], in_=psW[("q0", tt)])
        proj_fc1("q", 1)
        rope_tt(3, 0, psKQ)
        proj_fc1("k", 1)
        rope_tt(1, 0, psKQ)
        ss_slice(2)
        proj_fc1("q", 2)
        proj_fc1("k", 2)
        ss_slice(3)
        # r_tok via PE transposes of the r row
        rtok_ps = psKQ.tile([128, 16], f32, tag="proj", name="rtokps")
        for i in range(16):
            nc.tensor.transpose(
                rtok_ps[:, i : i + 1], r_sb[0:1, i * 128 : (i + 1) * 128],
                one_f32,
            )
        nc.vector.tensor_copy(out=r_tok, in_=rtok_ps)
        proj_fc1("q", 3)
        proj_fc1("k", 3)

        # ---- attention: scores [k,q] -> paired exp -> transposed AV ----
        # Quarter 0 runs INSIDE the psKQ ring (overlapping the QKV tail);
        # quarters 1-3 use dedicated pools: sc ring (3x2 banks, also V/pp/
        # outproj/avT) + av4 ring (2 banks).
        state = {"pending_oq": None, "sc_pool": psKQ, "split_sc": True,
                 "pending_proj": [("k", 2), ("q", 2), ("k", 3), ("q", 3)]}

        def v_proj(tt):
            pool = state["sc_pool"]
            tg = "proj" if pool is psKQ else "sc"
            psv = pool.tile([128, 256], f32, tag=tg, name=f"v_{tt}")
            for kc in range(KC):
                nc.tensor.matmul(
                    psv,
                    lhsT=xT_sb[:, kc, tt * 128 : (tt + 1) * 128],
                    rhs=wv_sb[:, kc, :],
                    start=(kc == 0),
                    stop=(kc == KC - 1),
                )
            nc.vector.tensor_scalar(
                out=v_sb[:, tt, :, 0:64],
                in0=psv.rearrange("p (h d) -> p h d", h=HPC),
                scalar1=r_tok[:, tt : tt + 1],
                scalar2=None,
                op0=mult,
            )

        def emit_outproj_do(qtp, do):
            pool = state["sc_pool"]
            tg = "proj" if pool is psKQ else "sc"
            po = pool.tile([128, 512], f32, tag=tg, name=f"o_{qtp}_{do}")
            for fc in range(2):
                nc.tensor.matmul(
                    po,
                    lhsT=wo_sb[:, fc, do * 128 : (do + 1) * 128],
                    rhs=av_all[:, fc, qtp * 512 : (qtp + 1) * 512],
                    start=(fc == 0),
                    stop=(fc == 1),
                )
            ob = work.tile([128, 512], bf16, tag="ob")
            if qtp == 3:
                nc.scalar.copy(out=ob, in_=po)  # Act is idle in the tail
            else:
                nc.vector.tensor_copy(out=ob, in_=po)
            nc.sync.dma_start(
                out.rearrange("(do p) t -> p do t", p=128)[
                    :, do, qtp * 512 : (qtp + 1) * 512
                ],
                ob,
            )

        def run_quarter(qt, av_pool):
            q0 = qt * 512
            pool = state["sc_pool"]
            tg = "proj" if pool is psKQ else "sc"
            split = state["split_sc"]
            for pi in range(2):
                # full-bank tiles: matmul start=True zeroes the whole 2KB
                # bank, so only the FIRST write into each bank uses it
                av4 = [
                    av_pool.tile(
                        [128, 4, 128], f32,
                        tag="proj" if av_pool is psKQ else "av4",
                        name=f"av_{qt}_{pi}_{x}",
                    )
                    for x in range(2)
                ]

                def emit_av(kb, ex):
                    for qbl in range(4):
                        qb = 4 * qt + qbl
                        if kb > qb:
                            continue
                        for x in range(2):
                            nc.tensor.matmul(
                                av4[x][:, qbl, 0:65],
                                lhsT=ex[:, x * 512 + qbl * 128 : x * 512 + (qbl + 1) * 128],
                                rhs=v_sb[:, kb, 2 * pi + x, :],
                                start=(kb == 0 and qbl == 0),
                                stop=(kb == qb),
                                skip_group_check=True,
                            )

                nkb = 4 * qt + 4
                pend = None
                for kb in range(nkb + 1):
                    cur = None
                    if kb < nkb:
                        c0 = max(0, kb * 128 - q0)
                        if split:
                            scs = [
                                pool.tile([128, 512], f32, tag=tg,
                                          name=f"sc_{qt}_{pi}_{kb}_{x}")
                                for x in range(2)
                            ]
                        else:
                            scp = pool.tile([128, 1024], f32, tag=tg,
                                            name=f"sc_{qt}_{pi}_{kb}")
                            scs = [scp[:, 0:512], scp[:, 512:1024]]
                        for x in range(2):
                            rX = slice(x * 64, x * 64 + 64)
                            nc.tensor.matmul(
                                scs[x][:, c0:512],
                                lhsT=qk_sb[rX, 2 + pi, kb * 128 : (kb + 1) * 128],
                                rhs=qk_sb[rX, pi, q0 + c0 : q0 + 512],
                                start=True,
                                stop=True,
                            )
                        if pi == 0 and kb < 4:
                            v_proj(4 * qt + kb)
                        # rope the NEXT quarter's token slice, one projection
                        # per kb iteration of pair 1
                        if pi == 1 and qt < 3 and kb < 4:
                            rope_tt((2, 0, 3, 1)[kb], qt + 1, pool)

                        if pi == 0 and state["pending_oq"] is not None and kb >= 2:
                            qtp, nd = state["pending_oq"]
                            todo = 8 - nd
                            left = nkb - kb
                            n_emit = -(-todo // max(left, 1))
                            for _ in range(min(n_emit, todo)):
                                emit_outproj_do(qtp, nd)
                                nd += 1
                            state["pending_oq"] = (qtp, nd) if nd < 8 else None
                        ex = expp.tile([128, 1024], bf16, tag="exp")
                        if split or c0 > 0:
                            for x in range(2):
                                nc.scalar.activation(
                                    ex[:, x * 512 + c0 : x * 512 + 512],
                                    scs[x][:, c0:512],
                                    Exp,
                                    scale=r_tok[:, kb : kb + 1],
                                )
                        else:
                            nc.scalar.activation(
                                ex, scp, Exp, scale=r_tok[:, kb : kb + 1]
                            )
                        if kb >= 4 * qt:  # diagonal block: causal mask
                            for x in range(2):
                                nc.gpsimd.tensor_tensor(
                                    ex[:, x * 512 + c0 : x * 512 + c0 + 128],
                                    ex[:, x * 512 + c0 : x * 512 + c0 + 128],
                                    mask_sb,
                                    mult,
                                )
                        cur = (kb, ex)
                    if pend is not None:
                        emit_av(*pend)
                    pend = cur
                # normalize (rows 0..63 / row 64) into token-major av_tok
                rec4s = []
                for x in range(2):
                    rec4 = recp.tile([128, 4], f32, tag="rec")
                    nc.vector.reciprocal(rec4, av4[x][:, :, 64:65])
                    rec4s.append(rec4)
                for qbl in range(4):
                    for x in range(2):
                        h = 2 * pi + x
                        nc.vector.tensor_scalar(
                            out=av_tok[:, 4 * qt + qbl, h * 64 : (h + 1) * 64],
                            in0=av4[x][:, qbl, 0:64],
                            scalar1=rec4s[x][:, qbl : qbl + 1],
                            scalar2=None,
                            op0=mult,
                        )
            # back to feature-major via PE transposes (53ns each)
            avT = pool.tile([128, 8, 128], bf16, tag=tg, name=f"avt_{qt}")
            for j, tt in enumerate(range(4 * qt, 4 * qt + 4)):
                for fc in range(2):
                    nc.tensor.transpose(
                        avT[:, fc * 4 + j, :],
                        av_tok[:, tt, fc * 128 : (fc + 1) * 128],
                        id_sb,
                    )
            for fc in range(2):
                nc.vector.tensor_copy(
                    out=av_all[:, fc, q0 : q0 + 512],
                    in_=avT[:, fc * 4 : fc * 4 + 4, :],
                )
            if qt < 3:
                # out-projection deferred into the next quarter's kb loop
                state["pending_oq"] = (qt, 0)
            else:
                for do in range(8):
                    emit_outproj_do(3, do)

        if QT0_IN_RING:
            # quarter 0 inside the psKQ ring, overlapping the QKV tail
            run_quarter(0, psKQ)
        ctxA.close()
        with (
            tc.tile_pool(name="psSC", bufs=3, space="PSUM") as psSC,
            tc.tile_pool(name="psAV", bufs=2, space="PSUM") as psAV,
        ):
            state["sc_pool"] = psSC
            state["split_sc"] = False
            for qt in range(0 if not QT0_IN_RING else 1, 4):
                run_quarter(qt, psAV)
            if tap == "rtok":
                nc.sync.dma_start(dbg[:, :], r_tok)
            elif tap == "qk":
                nc.sync.dma_start(dbg[:, :, :], qk_sb)
            elif tap == "v":
                nc.sync.dma_start(dbg[:, :, :, :], v_sb)
            elif tap == "avtok":
                nc.sync.dma_start(dbg[:, :, :], av_tok)
            elif tap == "avall":
                nc.sync.dma_start(dbg[:, :, :], av_all)
    nc.compile()
    return nc


def _host_inputs(x, norm_w, w_qkv, w_o, sin, cos):
    """Build the 8 per-core input maps (all bf16)."""
    n = T
    w_eff = np.asarray(w_qkv, np.float64) * np.asarray(norm_w, np.float64)[:, None]
    sin_n = np.asarray(sin, np.float32)[:n]  # [T, 64]
    cos_n = np.asarray(cos, np.float32)[:n]
    sign = np.concatenate([-np.ones(32, np.float32), np.ones(32, np.float32)])
    cos_tile = np.tile(cos_n.T, (2, 1))  # [128, T]
    sin_tile = np.tile((sin_n * sign[None, :]).T, (2, 1))  # [128, T]
    perm = np.zeros((128, 128), np.float32)
    for m in range(128):
        d = m % 64
        k = m + 32 if d < 32 else m - 32
        perm[k, m] = 1.0
    ident_np = np.eye(128, dtype=np.float32)
    ql = np.arange(128)[None, :]
    key = np.arange(128)[:, None]
    masks = (ql >= key).astype(np.float32)

    in_maps = []
    for c in range(8):
        b, g = c // 4, c % 4
        fs = slice(g * F, (g + 1) * F)
        xT_np = np.ascontiguousarray(np.asarray(x, np.float32)[b].T).astype(BF16)
        in_maps.append(
            {
                "xT": xT_np,
                "wq": (w_eff[:, 0:DIM][:, fs] * (DIM_HEAD ** -0.5)).astype(BF16),
                "wk": w_eff[:, DIM : 2 * DIM][:, fs].astype(BF16),
                "wv": w_eff[:, 2 * DIM : 3 * DIM][:, fs].astype(BF16),
                "wo": np.asarray(w_o, np.float32)[fs, :].astype(BF16),
                "cosT": cos_tile.astype(BF16),
                "sinT": sin_tile.astype(BF16),
                "perm": perm.astype(BF16),
                "masks": masks.astype(BF16),
                "ident": ident_np.astype(BF16),
            }
        )
    return in_maps


def kernel(x, norm_w, w_qkv, w_o, b_o, sin, cos):
    from concourse.bass_utils import run_bass_kernel_spmd

    if "nc" not in _NC_CACHE:
        _NC_CACHE["nc"] = _build_nc()
    nc = _NC_CACHE["nc"]
    in_maps = _host_inputs(x, norm_w, w_qkv, w_o, sin, cos)
    trace = bool(int(os.environ.get("KERNEL_TRACE", "0")))
    res = run_bass_kernel_spmd(nc, in_maps, core_ids=list(range(8)), trace=trace)
    if trace and res.exec_time_ns is not None:
        print(f"HW exec time: {res.exec_time_ns} ns")
    outs = [r["out"].astype(np.float32) for r in res.results]  # [1024, T] fm
    b_o = np.asarray(b_o, np.float32)
    full = np.empty((B, T, DIM), np.float32)
    for b in range(B):
        acc = outs[b * 4] + outs[b * 4 + 1] + outs[b * 4 + 2] + outs[b * 4 + 3]
        full[b] = acc.T + b_o[None, :]
    return full


# revision 89
# speedup vs baseline: 1.1160x; 1.0032x over previous
"""Trainium2 8-core kernel for RMSNorm -> QKV -> RoPE -> causal SDPA -> out-proj.

Sharding: core c = b*4 + g handles batch b (of 2) and heads 4g..4g+3 (of 16).
Each core computes a partial out-projection [dim, tokens]; the host sums the
4 head-group partials per batch (the tensor-parallel "unshard") and adds b_o.

Cost-model-driven layout (TimelineSim charges matmuls by OUTPUT FREE SIZE
only — contraction depth and output partitions are free):
  - scores per (head, kb): [key 128, q free] trimmed to the causal triangle.
  - AV runs TRANSPOSED: out [q 128, d 65] so each accumulation step costs 65
    rows instead of ~512; the ones column gives the softmax denominator.
    (matmul start=True zeroes the whole 2KB PSUM bank, so av4 tiles are
    bank-sized and only the first write uses start=True.)
  - The normalized token-major AV result returns to feature-major layout via
    PE transposes (53ns per 128x128 block) + one DVE copy per fc half.
  - exp for a head PAIR is fused into one Activation instruction (the two
    heads' score tiles sit in adjacent PSUM banks); Sqrt/Exp act tables are
    preloaded at t=0 with dummy activations.
  - r = rsqrt(mean x^2) rides into Q via r-scaled RoPE tables, into scores
    via the per-key `scale` operand of exp, and into V via a per-partition
    tensor_scalar during the PSUM->SBUF copy. r_tok (token-major r) comes
    from 16 free PE transposes of the r row.
  - Latency hiding: K/Q-fc0 project together chunk-paced off the DMA stream
    (PE idle gaps halve PE speed until 3us of continuous execution); the
    ss/r-chain, K/Q-fc1 and the tt0 RoPE interleave in wave 2; V projection,
    next-quarter RoPE, and the previous quarter's out-projection are all
    spread through the attention kb loops so the scalar engine (exp) stays
    fed; the last quarter's out-proj copies ride the then-idle Act engine.
"""

import os

import numpy as np
import ml_dtypes

BF16 = ml_dtypes.bfloat16

DIM = 1024
HEADS = 16
DIM_HEAD = 64
T = 2048  # tokens per batch
B = 2
HPC = 4  # heads per core
F = HPC * DIM_HEAD  # 256 per-core head width
KC = DIM // 128  # 8 contraction chunks
KORD = [2, 3, 4, 5, 6, 7, 0, 1]  # kc order: first matmul waits for chunk 2
TAIL_FINE = int(os.environ.get("KTAIL", "0"))
QT0_IN_RING = int(os.environ.get("KQT0", "0"))

_NC_CACHE = {}


def _build_nc():
    import concourse.bacc as bacc
    import concourse.mybir as mybir
    import concourse.tile as tile
    from contextlib import ExitStack

    f32 = mybir.dt.float32
    bf16 = mybir.dt.bfloat16
    nc = bacc.Bacc()

    xT = nc.declare_dram_parameter("xT", [DIM, T], bf16, isOutput=False)
    wq = nc.declare_dram_parameter("wq", [2 * 128, KC * 128], bf16, isOutput=False)
    wk = nc.declare_dram_parameter("wk", [2 * 128, KC * 128], bf16, isOutput=False)
    wv = nc.declare_dram_parameter("wv", [DIM, F], bf16, isOutput=False)
    wo = nc.declare_dram_parameter("wo", [F, DIM], bf16, isOutput=False)
    cosT = nc.declare_dram_parameter("cosT", [128, T], bf16, isOutput=False)
    sinT = nc.declare_dram_parameter("sinT", [128, T], bf16, isOutput=False)
    perm = nc.declare_dram_parameter("perm", [128, 128], bf16, isOutput=False)
    masks = nc.declare_dram_parameter("masks", [128, 128], bf16, isOutput=False)
    ident = nc.declare_dram_parameter("ident", [128, 128], bf16, isOutput=False)
    out = nc.declare_dram_parameter("out", [DIM, T], bf16, isOutput=True)
    tap = os.environ.get("KTAP", "")
    dbg = None
    if tap:
        _tap_shapes = {
            "rtok": ([128, 16], f32),
            "qk": ([128, 4, T], bf16),
            "v": ([128, 16, HPC, 65], bf16),
            "avtok": ([128, 16, F], bf16),
            "avall": ([128, 2, T], bf16),
        }
        shp, dt = _tap_shapes[tap]
        dbg = nc.declare_dram_parameter("dbg", shp, dt, isOutput=True)

    Exp = mybir.ActivationFunctionType.Exp
    Sqrt = mybir.ActivationFunctionType.Sqrt
    mult = mybir.AluOpType.mult
    add = mybir.AluOpType.add

    with ExitStack() as ctx:
        tc = ctx.enter_context(tile.TileContext(nc))
        consts = ctx.enter_context(tc.tile_pool(name="consts", bufs=1))
        persist = ctx.enter_context(tc.tile_pool(name="persist", bufs=1))
        work = ctx.enter_context(tc.tile_pool(name="work", bufs=4))
        vecs = ctx.enter_context(tc.tile_pool(name="vecs", bufs=1))

        # ---- constants / inputs ----
        wk_sb = consts.tile([128, 2, KC, 128], bf16, tag="wk")
        wq_sb = consts.tile([128, 2, KC, 128], bf16, tag="wq")
        wv_sb = consts.tile([128, KC, F], bf16, tag="wv")
        wo_sb = consts.tile([128, 2, DIM], bf16, tag="wo")
        cos_sb = consts.tile([128, T], bf16, tag="cos")
        sin_sb = consts.tile([128, T], bf16, tag="sin")
        perm_sb = consts.tile([128, 128], bf16, tag="perm")
        mask_sb = consts.tile([128, 128], bf16, tag="mask")
        id_sb = consts.tile([128, 128], bf16, tag="ident")
        ones_col = consts.tile([128, 1], bf16, tag="onesc")
        one_f32 = consts.tile([1, 1], f32, tag="onef")
        xT_sb = persist.tile([128, KC, T], bf16, tag="xT")
        xT_r = xT.rearrange("(kc p) t -> p kc t", p=128)
        # wk first (first PE consumer), then xT chunks in consumption order
        # with the other weights slotted behind the early chunks
        wk_r = wk.rearrange("(fc p) (kc d) -> p fc kc d", p=128, d=128)
        wq_r = wq.rearrange("(fc p) (kc d) -> p fc kc d", p=128, d=128)
        nc.sync.dma_start(wk_sb[:, 0], wk_r[:, 0])
        nc.sync.dma_start(wq_sb[:, 0], wq_r[:, 0])
        for kc in KORD[:4]:
            nc.sync.dma_start(xT_sb[:, kc], xT_r[:, kc])
        nc.sync.dma_start(wk_sb[:, 1], wk_r[:, 1])
        nc.sync.dma_start(wq_sb[:, 1], wq_r[:, 1])
        nc.sync.dma_start(perm_sb, perm[:, :])
        nc.sync.dma_start(cos_sb, cosT[:, :])
        nc.sync.dma_start(sin_sb, sinT[:, :])
        for kc in KORD[4:]:
            nc.sync.dma_start(xT_sb[:, kc], xT_r[:, kc])
        nc.sync.dma_start(wv_sb, wv.rearrange("(kc p) f -> p kc f", p=128))
        nc.sync.dma_start(mask_sb, masks[:, :])
        nc.sync.dma_start(id_sb, ident[:, :])
        nc.sync.dma_start(wo_sb, wo.rearrange("(fc p) d -> p fc d", p=128))
        nc.vector.memset(ones_col, 1.0)
        nc.vector.memset(one_f32, 1.0)

        # persistent activations
        qk_sb = persist.tile([128, 4, T], bf16, tag="qk")  # 0,1=q fc0/1; 2,3=k
        v_sb = persist.tile([128, 16, HPC, 65], bf16, tag="v")
        av_tok = persist.tile([128, 16, F], bf16, tag="avtok")
        av_all = persist.tile([128, 2, T], bf16, tag="av")
        r_sb = vecs.tile([1, T], f32, tag="r")
        r_tok = vecs.tile([128, 16], f32, tag="rtok")
        r_bc = persist.tile([128, T], f32, tag="rbc")
        cosr_sb = persist.tile([128, T], bf16, tag="cosr")
        sinr_sb = persist.tile([128, T], bf16, tag="sinr")
        qraw_sb = persist.tile([128, 2, 4, 512], bf16, tag="qraw")
        kraw_sb = persist.tile([128, 2, 4, 512], bf16, tag="kraw")
        nc.vector.memset(v_sb[:, :, :, 64:65], 1.0)
        # preload the Sqrt/Exp activation tables while DMAs stream in
        dum = vecs.tile([1, 1], f32, tag="dum")
        nc.scalar.activation(dum, one_f32, Sqrt)
        nc.scalar.activation(dum, dum, Exp)

        expp = ctx.enter_context(tc.tile_pool(name="expp", bufs=8))
        recp = ctx.enter_context(tc.tile_pool(name="recp", bufs=4))

        ctxA = ExitStack()
        psKQ = ctxA.enter_context(tc.tile_pool(name="psKQ", bufs=8, space="PSUM"))
        sbA = ctxA.enter_context(tc.tile_pool(name="sbA", bufs=1))
        xsq_sb = sbA.tile([128, KC, T], bf16, tag="xsq")

        # x^2 per chunk (DVE, chases the xT DMAs)
        for kc in KORD:
            nc.vector.tensor_mul(xsq_sb[:, kc], xT_sb[:, kc], xT_sb[:, kc])

        def rope_tt(fidx, tt, pool):
            """RoPE one 512-token slice of Q/K from the raw SBUF copy:
            rotate-half perm matmul + two multiplies + add into qk_sb.
            Q (fidx 0,1) uses the r-scaled tables so r_q rides in free."""
            ts = slice(tt * 512, (tt + 1) * 512)
            is_q = fidx < 2
            raw = (qraw_sb if is_q else kraw_sb)[:, fidx % 2, tt]
            cc = cosr_sb if is_q else cos_sb
            ssb = sinr_sb if is_q else sin_sb
            pp = pool.tile([128, 512], f32, tag="sc" if pool is not psKQ else "proj",
                           name=f"pp_{fidx}_{tt}")
            nc.tensor.matmul(pp, lhsT=perm_sb, rhs=raw, start=True, stop=True)
            t1 = work.tile([128, 512], bf16, tag="t1")
            nc.vector.tensor_tensor(t1, pp, ssb[:, ts], mult)
            t2 = work.tile([128, 512], bf16, tag="t2")
            nc.vector.tensor_tensor(t2, raw, cc[:, ts], mult)
            nc.vector.tensor_tensor(qk_sb[:, fidx, ts], t2, t1, add)

        # ---- wave 1: K-fc0 + Q-fc0 projections, chunk-paced off DMA ----
        psW = {}
        for nm in ("k0", "q0"):
            for tt in range(4):
                psW[(nm, tt)] = psKQ.tile(
                    [128, 512], f32, tag="proj", name=f"{nm}_{tt}"
                )
        for kc in KORD:
            for tt in range(4):
                ts = slice(tt * 512, (tt + 1) * 512)
                nc.tensor.matmul(
                    psW[("k0", tt)],
                    lhsT=wk_sb[:, 0, kc],
                    rhs=xT_sb[:, kc, ts],
                    start=(kc == KORD[0]),
                    stop=(kc == KORD[-1]),
                )
                nc.tensor.matmul(
                    psW[("q0", tt)],
                    lhsT=wq_sb[:, 0, kc],
                    rhs=xT_sb[:, kc, ts],
                    start=(kc == KORD[0]),
                    stop=(kc == KORD[-1]),
                )
        # free the k0 slots first (Act; DVE is still finishing x^2);
        # q0 copies are deferred into wave 2 so the r-chain starts sooner
        for tt in range(4):
            nc.scalar.copy(out=kraw_sb[:, 0, tt], in_=psW[("k0", tt)])

        # ---- wave 2: ss/r-chain + K-fc1 + Q-fc1 interleaved ----
        ss_sb = sbA.tile([1, T], f32, tag="ss")

        def proj_fc1(which, tt, pool=None):
            pool = pool if pool is not None else psKQ
            tg = "proj" if pool is psKQ else "sc"
            w = wk_sb if which == "k" else wq_sb
            psq = pool.tile([128, 512], f32, tag=tg, name=f"{which}1_{tt}")
            for kc in range(KC):
                nc.tensor.matmul(
                    psq,
                    lhsT=w[:, 1, kc],
                    rhs=xT_sb[:, kc, tt * 512 : (tt + 1) * 512],
                    start=(kc == 0),
                    stop=(kc == KC - 1),
                )
            if which == "k":
                nc.vector.tensor_copy(out=kraw_sb[:, 1, tt], in_=psq)
            else:
                nc.scalar.copy(out=qraw_sb[:, 1, tt], in_=psq)

        def ss_slice(s):
            ts = slice(s * 512, (s + 1) * 512)
            ss_ps = psKQ.tile([1, 512], f32, tag="proj", name=f"ss_{s}")
            for kc in range(KC):
                nc.tensor.matmul(
                    ss_ps,
                    lhsT=ones_col,
                    rhs=xsq_sb[:, kc, s * 512 : (s + 1) * 512],
                    start=(kc == 0),
                    stop=(kc == KC - 1),
                )
            nc.scalar.activation(
                ss_sb[:, ts], ss_ps, Sqrt, scale=1.0 / DIM
            )
            nc.vector.reciprocal(r_sb[:, ts], ss_sb[:, ts])
            nc.gpsimd.partition_broadcast(r_bc[:, ts], r_sb[:, ts])
            nc.gpsimd.tensor_tensor(cosr_sb[:, ts], cos_sb[:, ts], r_bc[:, ts], mult)
            nc.gpsimd.tensor_tensor(sinr_sb[:, ts], sin_sb[:, ts], r_bc[:, ts], mult)

        proj_fc1("q", 0)
        ss_slice(0)
        for tt in (0, 1):
            nc.scalar.copy(out=qraw_sb[:, 0, tt], in_=psW[("q0", tt)])
        rope_tt(2, 0, psKQ)
        proj_fc1("k", 0)
        ss_slice(1)
        rope_tt(0, 0, psKQ)
        for tt in (2, 3):
            nc.scalar.copy(out=qraw_sb[:, 0, tt], in_=psW[("q0", tt)])
        proj_fc1("q", 1)
        rope_tt(3, 0, psKQ)
        proj_fc1("k", 1)
        rope_tt(1, 0, psKQ)
        ss_slice(2)
        proj_fc1("q", 2)
        proj_fc1("k", 2)
        ss_slice(3)
        # r_tok via PE transposes of the r row
        rtok_ps = psKQ.tile([128, 16], f32, tag="proj", name="rtokps")
        for i in range(16):
            nc.tensor.transpose(
                rtok_ps[:, i : i + 1], r_sb[0:1, i * 128 : (i + 1) * 128],
                one_f32,
            )
        nc.vector.tensor_copy(out=r_tok, in_=rtok_ps)
        proj_fc1("q", 3)
        proj_fc1("k", 3)

        # ---- attention: scores [k,q] -> paired exp -> transposed AV ----
        # Quarter 0 runs INSIDE the psKQ ring (overlapping the QKV tail);
        # quarters 1-3 use dedicated pools: sc ring (3x2 banks, also V/pp/
        # outproj/avT) + av4 ring (2 banks).
        state = {"pending_oq": None, "sc_pool": psKQ, "split_sc": True,
                 "pending_proj": [("k", 2), ("q", 2), ("k", 3), ("q", 3)]}

        def v_proj(tt):
            pool = state["sc_pool"]
            tg = "proj" if pool is psKQ else "sc"
            psv = pool.tile([128, 256], f32, tag=tg, name=f"v_{tt}")
            for kc in range(KC):
                nc.tensor.matmul(
                    psv,
                    lhsT=xT_sb[:, kc, tt * 128 : (tt + 1) * 128],
                    rhs=wv_sb[:, kc, :],
                    start=(kc == 0),
                    stop=(kc == KC - 1),
                )
            nc.vector.tensor_scalar(
                out=v_sb[:, tt, :, 0:64],
                in0=psv.rearrange("p (h d) -> p h d", h=HPC),
                scalar1=r_tok[:, tt : tt + 1],
                scalar2=None,
                op0=mult,
            )

        def emit_outproj_do(qtp, do):
            pool = state["sc_pool"]
            tg = "proj" if pool is psKQ else "sc"
            po = pool.tile([128, 512], f32, tag=tg, name=f"o_{qtp}_{do}")
            for fc in range(2):
                nc.tensor.matmul(
                    po,
                    lhsT=wo_sb[:, fc, do * 128 : (do + 1) * 128],
                    rhs=av_all[:, fc, qtp * 512 : (qtp + 1) * 512],
                    start=(fc == 0),
                    stop=(fc == 1),
                )
            ob = work.tile([128, 512], bf16, tag="ob")
            if qtp == 3:
                nc.scalar.copy(out=ob, in_=po)  # Act is idle in the tail
            else:
                nc.vector.tensor_copy(out=ob, in_=po)
            nc.sync.dma_start(
                out.rearrange("(do p) t -> p do t", p=128)[
                    :, do, qtp * 512 : (qtp + 1) * 512
                ],
                ob,
            )

        def run_quarter(qt, av_pool):
            q0 = qt * 512
            pool = state["sc_pool"]
            tg = "proj" if pool is psKQ else "sc"
            split = state["split_sc"]
            for pi in range(2):
                # full-bank tiles: matmul start=True zeroes the whole 2KB
                # bank, so only the FIRST write into each bank uses it
                av4 = [
                    av_pool.tile(
                        [128, 4, 128], f32,
                        tag="proj" if av_pool is psKQ else "av4",
                        name=f"av_{qt}_{pi}_{x}",
                    )
                    for x in range(2)
                ]

                def emit_av(kb, ex):
                    for qbl in range(4):
                        qb = 4 * qt + qbl
                        if kb > qb:
                            continue
                        for x in range(2):
                            nc.tensor.matmul(
                                av4[x][:, qbl, 0:65],
                                lhsT=ex[:, x * 512 + qbl * 128 : x * 512 + (qbl + 1) * 128],
                                rhs=v_sb[:, kb, 2 * pi + x, :],
                                start=(kb == 0 and qbl == 0),
                                stop=(kb == qb),
                                skip_group_check=True,
                            )

                nkb = 4 * qt + 4
                pend = None
                for kb in range(nkb + 1):
                    cur = None
                    if kb < nkb:
                        c0 = max(0, kb * 128 - q0)
                        if split:
                            scs = [
                                pool.tile([128, 512], f32, tag=tg,
                                          name=f"sc_{qt}_{pi}_{kb}_{x}")
                                for x in range(2)
                            ]
                        else:
                            scp = pool.tile([128, 1024], f32, tag=tg,
                                            name=f"sc_{qt}_{pi}_{kb}")
                            scs = [scp[:, 0:512], scp[:, 512:1024]]
                        for x in range(2):
                            rX = slice(x * 64, x * 64 + 64)
                            nc.tensor.matmul(
                                scs[x][:, c0:512],
                                lhsT=qk_sb[rX, 2 + pi, kb * 128 : (kb + 1) * 128],
                                rhs=qk_sb[rX, pi, q0 + c0 : q0 + 512],
                                start=True,
                                stop=True,
                            )
                        if pi == 0 and kb < 4:
                            v_proj(4 * qt + kb)
                        # rope the NEXT quarter's token slice, one projection
                        # per kb iteration of pair 1
                        if pi == 1 and qt < 3 and kb < 4:
                            rope_tt((2, 0, 3, 1)[kb], qt + 1, pool)

                        if pi == 0 and state["pending_oq"] is not None and kb >= 2:
                            qtp, nd = state["pending_oq"]
                            todo = 8 - nd
                            left = nkb - kb
                            n_emit = -(-todo // max(left, 1))
                            for _ in range(min(n_emit, todo)):
                                emit_outproj_do(qtp, nd)
                                nd += 1
                            state["pending_oq"] = (qtp, nd) if nd < 8 else None
                        ex = expp.tile([128, 1024], bf16, tag="exp")
                        if split or c0 > 0:
                            for x in range(2):
                                nc.scalar.activation(
                                    ex[:, x * 512 + c0 : x * 512 + 512],
                                    scs[x][:, c0:512],
                                    Exp,
                                    scale=r_tok[:, kb : kb + 1],
                                )
                        else:
                            nc.scalar.activation(
                                ex, scp, Exp, scale=r_tok[:, kb : kb + 1]
                            )
                        if kb >= 4 * qt:  # diagonal block: causal mask
                            for x in range(2):
                                nc.gpsimd.tensor_tensor(
                                    ex[:, x * 512 + c0 : x * 512 + c0 + 128],
                                    ex[:, x * 512 + c0 : x * 512 + c0 + 128],
                                    mask_sb,
                                    mult,
                                )
                        cur = (kb, ex)
                    if pend is not None:
                        emit_av(*pend)
                    pend = cur
                # normalize (rows 0..63 / row 64) into token-major av_tok
                rec4s = []
                for x in range(2):
                    rec4 = recp.tile([128, 4], f32, tag="rec")
                    nc.vector.reciprocal(rec4, av4[x][:, :, 64:65])
                    rec4s.append(rec4)
                for qbl in range(4):
                    for x in range(2):
                        h = 2 * pi + x
                        nc.vector.tensor_scalar(
                            out=av_tok[:, 4 * qt + qbl, h * 64 : (h + 1) * 64],
                            in0=av4[x][:, qbl, 0:64],
                            scalar1=rec4s[x][:, qbl : qbl + 1],
                            scalar2=None,
                            op0=mult,
                        )
            # back to feature-major via PE transposes (53ns each)
            avT = pool.tile([128, 8, 128], bf16, tag=tg, name=f"avt_{qt}")
            for j, tt in enumerate(range(4 * qt, 4 * qt + 4)):
                for fc in range(2):
                    nc.tensor.transpose(
                        avT[:, fc * 4 + j, :],
                        av_tok[:, tt, fc * 128 : (fc + 1) * 128],
                        id_sb,
                    )
            for fc in range(2):
                nc.vector.tensor_copy(
                    out=av_all[:, fc, q0 : q0 + 512],
                    in_=avT[:, fc * 4 : fc * 4 + 4, :],
                )
            if qt < 3:
                # out-projection deferred into the next quarter's kb loop
                state["pending_oq"] = (qt, 0)
            else:
                for do in range(8):
                    emit_outproj_do(3, do)

        if QT0_IN_RING:
            # quarter 0 inside the psKQ ring, overlapping the QKV tail
            run_quarter(0, psKQ)
        ctxA.close()
        with (
            tc.tile_pool(name="psSC", bufs=3, space="PSUM") as psSC,
            tc.tile_pool(name="psAV", bufs=2, space="PSUM") as psAV,
        ):
            state["sc_pool"] = psSC
            state["split_sc"] = False
            for qt in range(0 if not QT0_IN_RING else 1, 4):
                run_quarter(qt, psAV)
            if tap == "rtok":
                nc.sync.dma_start(dbg[:, :], r_tok)
            elif tap == "qk":
                nc.sync.dma_start(dbg[:, :, :], qk_sb)
            elif tap == "v":
                nc.sync.dma_start(dbg[:, :, :, :], v_sb)
            elif tap == "avtok":
                nc.sync.dma_start(dbg[:, :, :], av_tok)
            elif tap == "avall":
                nc.sync.dma_start(dbg[:, :, :], av_all)
    nc.compile()
    return nc


def _fc_major(w):
    """[1024, 256] -> [2*128, 8*128]: row fc*128+p, cols kc*128+d (matches
    the [128, 2, KC, 128] SBUF layout loaded with a straight DMA)."""
    return np.ascontiguousarray(
        np.asarray(w, np.float64).reshape(KC, 128, 2, 128)
        .transpose(2, 1, 0, 3).reshape(2 * 128, KC * 128)
    ).astype(BF16)


def _host_inputs(x, norm_w, w_qkv, w_o, sin, cos):
    """Build the 8 per-core input maps (all bf16)."""
    n = T
    w_eff = np.asarray(w_qkv, np.float64) * np.asarray(norm_w, np.float64)[:, None]
    sin_n = np.asarray(sin, np.float32)[:n]  # [T, 64]
    cos_n = np.asarray(cos, np.float32)[:n]
    sign = np.concatenate([-np.ones(32, np.float32), np.ones(32, np.float32)])
    cos_tile = np.tile(cos_n.T, (2, 1))  # [128, T]
    sin_tile = np.tile((sin_n * sign[None, :]).T, (2, 1))  # [128, T]
    perm = np.zeros((128, 128), np.float32)
    for m in range(128):
        d = m % 64
        k = m + 32 if d < 32 else m - 32
        perm[k, m] = 1.0
    ident_np = np.eye(128, dtype=np.float32)
    ql = np.arange(128)[None, :]
    key = np.arange(128)[:, None]
    masks = (ql >= key).astype(np.float32)

    in_maps = []
    for c in range(8):
        b, g = c // 4, c % 4
        fs = slice(g * F, (g + 1) * F)
        xT_np = np.ascontiguousarray(np.asarray(x, np.float32)[b].T).astype(BF16)
        in_maps.append(
            {
                "xT": xT_np,
                "wq": _fc_major(w_eff[:, 0:DIM][:, fs] * (DIM_HEAD ** -0.5)),
                "wk": _fc_major(w_eff[:, DIM : 2 * DIM][:, fs]),
                "wv": w_eff[:, 2 * DIM : 3 * DIM][:, fs].astype(BF16),
                "wo": np.asarray(w_o, np.float32)[fs, :].astype(BF16),
                "cosT": cos_tile.astype(BF16),
                "sinT": sin_tile.astype(BF16),
                "perm": perm.astype(BF16),
                "masks": masks.astype(BF16),
                "ident": ident_np.astype(BF16),
            }
        )
    return in_maps


def kernel(x, norm_w, w_qkv, w_o, b_o, sin, cos):
    from concourse.bass_utils import run_bass_kernel_spmd

    if "nc" not in _NC_CACHE:
        _NC_CACHE["nc"] = _build_nc()
    nc = _NC_CACHE["nc"]
    in_maps = _host_inputs(x, norm_w, w_qkv, w_o, sin, cos)
    trace = bool(int(os.environ.get("KERNEL_TRACE", "0")))
    res = run_bass_kernel_spmd(nc, in_maps, core_ids=list(range(8)), trace=trace)
    if trace and res.exec_time_ns is not None:
        print(f"HW exec time: {res.exec_time_ns} ns")
    outs = [r["out"].astype(np.float32) for r in res.results]  # [1024, T] fm
    b_o = np.asarray(b_o, np.float32)
    full = np.empty((B, T, DIM), np.float32)
    for b in range(B):
        acc = outs[b * 4] + outs[b * 4 + 1] + outs[b * 4 + 2] + outs[b * 4 + 3]
        full[b] = acc.T + b_o[None, :]
    return full


# revision 90
# speedup vs baseline: 1.1178x; 1.0017x over previous
"""Trainium2 8-core kernel for RMSNorm -> QKV -> RoPE -> causal SDPA -> out-proj.

Sharding: core c = b*4 + g handles batch b (of 2) and heads 4g..4g+3 (of 16).
Each core computes a partial out-projection [dim, tokens]; the host sums the
4 head-group partials per batch (the tensor-parallel "unshard") and adds b_o.

Cost-model-driven layout (TimelineSim charges matmuls by OUTPUT FREE SIZE
only — contraction depth and output partitions are free):
  - scores per (head, kb): [key 128, q free] trimmed to the causal triangle.
  - AV runs TRANSPOSED: out [q 128, d 65] so each accumulation step costs 65
    rows instead of ~512; the ones column gives the softmax denominator.
    (matmul start=True zeroes the whole 2KB PSUM bank, so av4 tiles are
    bank-sized and only the first write uses start=True.)
  - The normalized token-major AV result returns to feature-major layout via
    PE transposes (53ns per 128x128 block) + one DVE copy per fc half.
  - exp for a head PAIR is fused into one Activation instruction (the two
    heads' score tiles sit in adjacent PSUM banks); Sqrt/Exp act tables are
    preloaded at t=0 with dummy activations.
  - r = rsqrt(mean x^2) rides into Q via r-scaled RoPE tables, into scores
    via the per-key `scale` operand of exp, and into V via a per-partition
    tensor_scalar during the PSUM->SBUF copy. r_tok (token-major r) comes
    from 16 free PE transposes of the r row.
  - Latency hiding: K/Q-fc0 project together chunk-paced off the DMA stream
    (PE idle gaps halve PE speed until 3us of continuous execution); the
    ss/r-chain, K/Q-fc1 and the tt0 RoPE interleave in wave 2; V projection,
    next-quarter RoPE, and the previous quarter's out-projection are all
    spread through the attention kb loops so the scalar engine (exp) stays
    fed; the last quarter's out-proj copies ride the then-idle Act engine.
"""

import os

import numpy as np
import ml_dtypes

BF16 = ml_dtypes.bfloat16

DIM = 1024
HEADS = 16
DIM_HEAD = 64
T = 2048  # tokens per batch
B = 2
HPC = 4  # heads per core
F = HPC * DIM_HEAD  # 256 per-core head width
KC = DIM // 128  # 8 contraction chunks
KORD = [2, 3, 4, 5, 6, 7, 0, 1]  # kc order: first matmul waits for chunk 2
TAIL_FINE = int(os.environ.get("KTAIL", "0"))
QT0_IN_RING = int(os.environ.get("KQT0", "0"))

_NC_CACHE = {}


def _build_nc():
    import concourse.bacc as bacc
    import concourse.mybir as mybir
    import concourse.tile as tile
    from contextlib import ExitStack

    f32 = mybir.dt.float32
    bf16 = mybir.dt.bfloat16
    nc = bacc.Bacc()

    xT = nc.declare_dram_parameter("xT", [DIM, T], bf16, isOutput=False)
    wq = nc.declare_dram_parameter("wq", [2 * 128, KC * 128], bf16, isOutput=False)
    wk = nc.declare_dram_parameter("wk", [2 * 128, KC * 128], bf16, isOutput=False)
    wv = nc.declare_dram_parameter("wv", [DIM, F], bf16, isOutput=False)
    wo = nc.declare_dram_parameter("wo", [F, DIM], bf16, isOutput=False)
    cosT = nc.declare_dram_parameter("cosT", [128, T], bf16, isOutput=False)
    sinT = nc.declare_dram_parameter("sinT", [128, T], bf16, isOutput=False)
    perm = nc.declare_dram_parameter("perm", [128, 128], bf16, isOutput=False)
    masks = nc.declare_dram_parameter("masks", [128, 128], bf16, isOutput=False)
    ident = nc.declare_dram_parameter("ident", [128, 128], bf16, isOutput=False)
    out = nc.declare_dram_parameter("out", [DIM, T], bf16, isOutput=True)
    tap = os.environ.get("KTAP", "")
    dbg = None
    if tap:
        _tap_shapes = {
            "rtok": ([128, 16], f32),
            "qk": ([128, 4, T], bf16),
            "v": ([128, 16, HPC, 65], bf16),
            "avtok": ([128, 16, F], bf16),
            "avall": ([128, 2, T], bf16),
        }
        shp, dt = _tap_shapes[tap]
        dbg = nc.declare_dram_parameter("dbg", shp, dt, isOutput=True)

    Exp = mybir.ActivationFunctionType.Exp
    Sqrt = mybir.ActivationFunctionType.Sqrt
    mult = mybir.AluOpType.mult
    add = mybir.AluOpType.add

    with ExitStack() as ctx:
        tc = ctx.enter_context(tile.TileContext(nc))
        consts = ctx.enter_context(tc.tile_pool(name="consts", bufs=1))
        persist = ctx.enter_context(tc.tile_pool(name="persist", bufs=1))
        work = ctx.enter_context(tc.tile_pool(name="work", bufs=4))
        vecs = ctx.enter_context(tc.tile_pool(name="vecs", bufs=1))

        # ---- constants / inputs ----
        wk_sb = consts.tile([128, 2, KC, 128], bf16, tag="wk")
        wq_sb = consts.tile([128, 2, KC, 128], bf16, tag="wq")
        wv_sb = consts.tile([128, KC, F], bf16, tag="wv")
        wo_sb = consts.tile([128, 2, DIM], bf16, tag="wo")
        cos_sb = consts.tile([128, T], bf16, tag="cos")
        sin_sb = consts.tile([128, T], bf16, tag="sin")
        perm_sb = consts.tile([128, 128], bf16, tag="perm")
        mask_sb = consts.tile([128, 128], bf16, tag="mask")
        id_sb = consts.tile([128, 128], bf16, tag="ident")
        ones_col = consts.tile([128, 1], bf16, tag="onesc")
        one_f32 = consts.tile([1, 1], f32, tag="onef")
        xT_sb = persist.tile([128, KC, T], bf16, tag="xT")
        xT_r = xT.rearrange("(kc p) t -> p kc t", p=128)
        # wk first (first PE consumer), then xT chunks in consumption order
        # with the other weights slotted behind the early chunks
        wk_r = wk.rearrange("(fc p) (kc d) -> p fc kc d", p=128, d=128)
        wq_r = wq.rearrange("(fc p) (kc d) -> p fc kc d", p=128, d=128)
        nc.sync.dma_start(wk_sb[:, 0], wk_r[:, 0])
        nc.sync.dma_start(wq_sb[:, 0], wq_r[:, 0])
        for kc in KORD[:4]:
            nc.sync.dma_start(xT_sb[:, kc], xT_r[:, kc])
        nc.sync.dma_start(wk_sb[:, 1], wk_r[:, 1])
        nc.sync.dma_start(wq_sb[:, 1], wq_r[:, 1])
        nc.sync.dma_start(perm_sb, perm[:, :])
        nc.sync.dma_start(cos_sb, cosT[:, :])
        nc.sync.dma_start(sin_sb, sinT[:, :])
        for kc in KORD[4:]:
            nc.sync.dma_start(xT_sb[:, kc], xT_r[:, kc])
        nc.sync.dma_start(wv_sb, wv.rearrange("(kc p) f -> p kc f", p=128))
        nc.sync.dma_start(mask_sb, masks[:, :])
        nc.sync.dma_start(id_sb, ident[:, :])
        nc.sync.dma_start(wo_sb, wo.rearrange("(fc p) d -> p fc d", p=128))
        nc.vector.memset(ones_col, 1.0)
        nc.vector.memset(one_f32, 1.0)

        # persistent activations
        qk_sb = persist.tile([128, 4, T], bf16, tag="qk")  # 0,1=q fc0/1; 2,3=k
        v_sb = persist.tile([128, 16, HPC, 65], bf16, tag="v")
        av_tok = persist.tile([128, 16, F], bf16, tag="avtok")
        av_all = persist.tile([128, 2, T], bf16, tag="av")
        r_sb = vecs.tile([1, T], f32, tag="r")
        r_tok = vecs.tile([128, 16], f32, tag="rtok")
        r_bc = persist.tile([128, T], f32, tag="rbc")
        cosr_sb = persist.tile([128, T], bf16, tag="cosr")
        sinr_sb = persist.tile([128, T], bf16, tag="sinr")
        qraw_sb = persist.tile([128, 2, 4, 512], bf16, tag="qraw")
        kraw_sb = persist.tile([128, 2, 4, 512], bf16, tag="kraw")
        nc.vector.memset(v_sb[:, :, :, 64:65], 1.0)
        # preload the Sqrt/Exp activation tables while DMAs stream in
        dum = vecs.tile([1, 1], f32, tag="dum")
        nc.scalar.activation(dum, one_f32, Sqrt)
        nc.scalar.activation(dum, dum, Exp)

        expp = ctx.enter_context(tc.tile_pool(name="expp", bufs=8))
        recp = ctx.enter_context(tc.tile_pool(name="recp", bufs=4))

        ctxA = ExitStack()
        psKQ = ctxA.enter_context(tc.tile_pool(name="psKQ", bufs=8, space="PSUM"))
        sbA = ctxA.enter_context(tc.tile_pool(name="sbA", bufs=1))
        xsq_sb = sbA.tile([128, KC, T], bf16, tag="xsq")

        # x^2 per chunk (DVE, chases the xT DMAs)
        for kc in KORD:
            nc.vector.tensor_mul(xsq_sb[:, kc], xT_sb[:, kc], xT_sb[:, kc])

        def rope_tt(fidx, tt, pool):
            """RoPE one 512-token slice of Q/K from the raw SBUF copy:
            rotate-half perm matmul + two multiplies + add into qk_sb.
            Q (fidx 0,1) uses the r-scaled tables so r_q rides in free."""
            ts = slice(tt * 512, (tt + 1) * 512)
            is_q = fidx < 2
            raw = (qraw_sb if is_q else kraw_sb)[:, fidx % 2, tt]
            cc = cosr_sb if is_q else cos_sb
            ssb = sinr_sb if is_q else sin_sb
            pp = pool.tile([128, 512], f32, tag="sc" if pool is not psKQ else "proj",
                           name=f"pp_{fidx}_{tt}")
            nc.tensor.matmul(pp, lhsT=perm_sb, rhs=raw, start=True, stop=True)
            t1 = work.tile([128, 512], bf16, tag="t1")
            nc.vector.tensor_tensor(t1, pp, ssb[:, ts], mult)
            t2 = work.tile([128, 512], bf16, tag="t2")
            nc.vector.tensor_tensor(t2, raw, cc[:, ts], mult)
            nc.vector.tensor_tensor(qk_sb[:, fidx, ts], t2, t1, add)

        # ---- wave 1: K-fc0 + Q-fc0 projections, chunk-paced off DMA ----
        psW = {}
        for nm in ("k0", "q0"):
            for tt in range(4):
                psW[(nm, tt)] = psKQ.tile(
                    [128, 512], f32, tag="proj", name=f"{nm}_{tt}"
                )
        for kc in KORD:
            for tt in range(4):
                ts = slice(tt * 512, (tt + 1) * 512)
                nc.tensor.matmul(
                    psW[("k0", tt)],
                    lhsT=wk_sb[:, 0, kc],
                    rhs=xT_sb[:, kc, ts],
                    start=(kc == KORD[0]),
                    stop=(kc == KORD[-1]),
                )
                nc.tensor.matmul(
                    psW[("q0", tt)],
                    lhsT=wq_sb[:, 0, kc],
                    rhs=xT_sb[:, kc, ts],
                    start=(kc == KORD[0]),
                    stop=(kc == KORD[-1]),
                )
        # free the k0 slots first (Act; DVE is still finishing x^2);
        # q0 copies are deferred into wave 2 so the r-chain starts sooner
        for tt in range(4):
            nc.scalar.copy(out=kraw_sb[:, 0, tt], in_=psW[("k0", tt)])

        # ---- wave 2: ss/r-chain + K-fc1 + Q-fc1 interleaved ----
        ss_sb = sbA.tile([1, T], f32, tag="ss")

        def proj_fc1(which, tt, pool=None):
            pool = pool if pool is not None else psKQ
            tg = "proj" if pool is psKQ else "sc"
            w = wk_sb if which == "k" else wq_sb
            psq = pool.tile([128, 512], f32, tag=tg, name=f"{which}1_{tt}")
            for kc in range(KC):
                nc.tensor.matmul(
                    psq,
                    lhsT=w[:, 1, kc],
                    rhs=xT_sb[:, kc, tt * 512 : (tt + 1) * 512],
                    start=(kc == 0),
                    stop=(kc == KC - 1),
                )
            if which == "k":
                nc.vector.tensor_copy(out=kraw_sb[:, 1, tt], in_=psq)
            else:
                nc.scalar.copy(out=qraw_sb[:, 1, tt], in_=psq)

        def ss_slice(s):
            ts = slice(s * 512, (s + 1) * 512)
            ss_ps = psKQ.tile([1, 512], f32, tag="proj", name=f"ss_{s}")
            for kc in range(KC):
                nc.tensor.matmul(
                    ss_ps,
                    lhsT=ones_col,
                    rhs=xsq_sb[:, kc, s * 512 : (s + 1) * 512],
                    start=(kc == 0),
                    stop=(kc == KC - 1),
                )
            nc.scalar.activation(
                ss_sb[:, ts], ss_ps, Sqrt, scale=1.0 / DIM
            )
            nc.vector.reciprocal(r_sb[:, ts], ss_sb[:, ts])
            nc.gpsimd.partition_broadcast(r_bc[:, ts], r_sb[:, ts])
            nc.gpsimd.tensor_tensor(cosr_sb[:, ts], cos_sb[:, ts], r_bc[:, ts], mult)
            nc.gpsimd.tensor_tensor(sinr_sb[:, ts], sin_sb[:, ts], r_bc[:, ts], mult)

        proj_fc1("q", 0)
        ss_slice(0)
        for tt in (0, 1):
            nc.scalar.copy(out=qraw_sb[:, 0, tt], in_=psW[("q0", tt)])
        rope_tt(2, 0, psKQ)
        proj_fc1("k", 0)
        ss_slice(1)
        rope_tt(0, 0, psKQ)
        for tt in (2, 3):
            nc.scalar.copy(out=qraw_sb[:, 0, tt], in_=psW[("q0", tt)])
        proj_fc1("q", 1)
        rope_tt(3, 0, psKQ)
        proj_fc1("k", 1)
        rope_tt(1, 0, psKQ)
        ss_slice(2)
        proj_fc1("q", 2)
        proj_fc1("k", 2)
        ss_slice(3)
        # r_tok via PE transposes of the r row
        rtok_ps = psKQ.tile([128, 16], f32, tag="proj", name="rtokps")
        for i in range(16):
            nc.tensor.transpose(
                rtok_ps[:, i : i + 1], r_sb[0:1, i * 128 : (i + 1) * 128],
                one_f32,
            )
        nc.vector.tensor_copy(out=r_tok, in_=rtok_ps)
        proj_fc1("q", 3)
        proj_fc1("k", 3)

        # ---- attention: scores [k,q] -> paired exp -> transposed AV ----
        # Quarter 0 runs INSIDE the psKQ ring (overlapping the QKV tail);
        # quarters 1-3 use dedicated pools: sc ring (3x2 banks, also V/pp/
        # outproj/avT) + av4 ring (2 banks).
        state = {"pending_oq": None, "sc_pool": psKQ, "split_sc": True,
                 "pending_proj": [("k", 2), ("q", 2), ("k", 3), ("q", 3)]}

        def v_proj(tt):
            pool = state["sc_pool"]
            tg = "proj" if pool is psKQ else "sc"
            psv = pool.tile([128, 256], f32, tag=tg, name=f"v_{tt}")
            for kc in range(KC):
                nc.tensor.matmul(
                    psv,
                    lhsT=xT_sb[:, kc, tt * 128 : (tt + 1) * 128],
                    rhs=wv_sb[:, kc, :],
                    start=(kc == 0),
                    stop=(kc == KC - 1),
                )
            nc.vector.tensor_scalar(
                out=v_sb[:, tt, :, 0:64],
                in0=psv.rearrange("p (h d) -> p h d", h=HPC),
                scalar1=r_tok[:, tt : tt + 1],
                scalar2=None,
                op0=mult,
            )

        def emit_outproj_do(qtp, do):
            pool = state["sc_pool"]
            tg = "proj" if pool is psKQ else "sc"
            po = pool.tile([128, 512], f32, tag=tg, name=f"o_{qtp}_{do}")
            for fc in range(2):
                nc.tensor.matmul(
                    po,
                    lhsT=wo_sb[:, fc, do * 128 : (do + 1) * 128],
                    rhs=av_all[:, fc, qtp * 512 : (qtp + 1) * 512],
                    start=(fc == 0),
                    stop=(fc == 1),
                )
            if qtp == 3:
                # stage into the dead kraw buffer; ship 2 merged DMAs so the
                # tail pays 2x625ns HWDGE instead of 8x
                nc.scalar.copy(out=kraw_sb[:, do // 4, do % 4, :], in_=po)
                if do % 4 == 3:
                    h = do // 4
                    nc.sync.dma_start(
                        out.rearrange("(do p) t -> p do t", p=128)[
                            :, h * 4 : h * 4 + 4, qtp * 512 : (qtp + 1) * 512
                        ],
                        kraw_sb[:, h],
                    )
            else:
                ob = work.tile([128, 512], bf16, tag="ob")
                nc.vector.tensor_copy(out=ob, in_=po)
                nc.sync.dma_start(
                    out.rearrange("(do p) t -> p do t", p=128)[
                        :, do, qtp * 512 : (qtp + 1) * 512
                    ],
                    ob,
                )

        def run_quarter(qt, av_pool):
            q0 = qt * 512
            pool = state["sc_pool"]
            tg = "proj" if pool is psKQ else "sc"
            split = state["split_sc"]
            for pi in range(2):
                # full-bank tiles: matmul start=True zeroes the whole 2KB
                # bank, so only the FIRST write into each bank uses it
                av4 = [
                    av_pool.tile(
                        [128, 4, 128], f32,
                        tag="proj" if av_pool is psKQ else "av4",
                        name=f"av_{qt}_{pi}_{x}",
                    )
                    for x in range(2)
                ]

                def emit_av(kb, ex):
                    for qbl in range(4):
                        qb = 4 * qt + qbl
                        if kb > qb:
                            continue
                        for x in range(2):
                            nc.tensor.matmul(
                                av4[x][:, qbl, 0:65],
                                lhsT=ex[:, x * 512 + qbl * 128 : x * 512 + (qbl + 1) * 128],
                                rhs=v_sb[:, kb, 2 * pi + x, :],
                                start=(kb == 0 and qbl == 0),
                                stop=(kb == qb),
                                skip_group_check=True,
                            )

                nkb = 4 * qt + 4
                pend = None
                for kb in range(nkb + 1):
                    cur = None
                    if kb < nkb:
                        c0 = max(0, kb * 128 - q0)
                        if split:
                            scs = [
                                pool.tile([128, 512], f32, tag=tg,
                                          name=f"sc_{qt}_{pi}_{kb}_{x}")
                                for x in range(2)
                            ]
                        else:
                            scp = pool.tile([128, 1024], f32, tag=tg,
                                            name=f"sc_{qt}_{pi}_{kb}")
                            scs = [scp[:, 0:512], scp[:, 512:1024]]
                        for x in range(2):
                            rX = slice(x * 64, x * 64 + 64)
                            nc.tensor.matmul(
                                scs[x][:, c0:512],
                                lhsT=qk_sb[rX, 2 + pi, kb * 128 : (kb + 1) * 128],
                                rhs=qk_sb[rX, pi, q0 + c0 : q0 + 512],
                                start=True,
                                stop=True,
                            )
                        if pi == 0 and kb < 4:
                            v_proj(4 * qt + kb)
                        # rope the NEXT quarter's token slice, one projection
                        # per kb iteration of pair 1
                        if pi == 1 and qt < 3 and kb < 4:
                            rope_tt((2, 0, 3, 1)[kb], qt + 1, pool)

                        if pi == 0 and state["pending_oq"] is not None and kb >= 2:
                            qtp, nd = state["pending_oq"]
                            todo = 8 - nd
                            left = nkb - kb
                            n_emit = -(-todo // max(left, 1))
                            for _ in range(min(n_emit, todo)):
                                emit_outproj_do(qtp, nd)
                                nd += 1
                            state["pending_oq"] = (qtp, nd) if nd < 8 else None
                        ex = expp.tile([128, 1024], bf16, tag="exp")
                        if split or c0 > 0:
                            for x in range(2):
                                nc.scalar.activation(
                                    ex[:, x * 512 + c0 : x * 512 + 512],
                                    scs[x][:, c0:512],
                                    Exp,
                                    scale=r_tok[:, kb : kb + 1],
                                )
                        else:
                            nc.scalar.activation(
                                ex, scp, Exp, scale=r_tok[:, kb : kb + 1]
                            )
                        if kb >= 4 * qt:  # diagonal block: causal mask
                            for x in range(2):
                                nc.gpsimd.tensor_tensor(
                                    ex[:, x * 512 + c0 : x * 512 + c0 + 128],
                                    ex[:, x * 512 + c0 : x * 512 + c0 + 128],
                                    mask_sb,
                                    mult,
                                )
                        cur = (kb, ex)
                    if pend is not None:
                        emit_av(*pend)
                    pend = cur
                # normalize (rows 0..63 / row 64) into token-major av_tok
                rec4s = []
                for x in range(2):
                    rec4 = recp.tile([128, 4], f32, tag="rec")
                    nc.vector.reciprocal(rec4, av4[x][:, :, 64:65])
                    rec4s.append(rec4)
                for qbl in range(4):
                    for x in range(2):
                        h = 2 * pi + x
                        nc.vector.tensor_scalar(
                            out=av_tok[:, 4 * qt + qbl, h * 64 : (h + 1) * 64],
                            in0=av4[x][:, qbl, 0:64],
                            scalar1=rec4s[x][:, qbl : qbl + 1],
                            scalar2=None,
                            op0=mult,
                        )
            # back to feature-major via PE transposes (53ns each)
            avT = pool.tile([128, 8, 128], bf16, tag=tg, name=f"avt_{qt}")
            for j, tt in enumerate(range(4 * qt, 4 * qt + 4)):
                for fc in range(2):
                    nc.tensor.transpose(
                        avT[:, fc * 4 + j, :],
                        av_tok[:, tt, fc * 128 : (fc + 1) * 128],
                        id_sb,
                    )
            for fc in range(2):
                nc.vector.tensor_copy(
                    out=av_all[:, fc, q0 : q0 + 512],
                    in_=avT[:, fc * 4 : fc * 4 + 4, :],
                )
            if qt < 3:
                # out-projection deferred into the next quarter's kb loop
                state["pending_oq"] = (qt, 0)
            else:
                for do in range(8):
                    emit_outproj_do(3, do)

        if QT0_IN_RING:
            # quarter 0 inside the psKQ ring, overlapping the QKV tail
            run_quarter(0, psKQ)
        ctxA.close()
        with (
            tc.tile_pool(name="psSC", bufs=3, space="PSUM") as psSC,
            tc.tile_pool(name="psAV", bufs=2, space="PSUM") as psAV,
        ):
            state["sc_pool"] = psSC
            state["split_sc"] = False
            for qt in range(0 if not QT0_IN_RING else 1, 4):
                run_quarter(qt, psAV)
            if tap == "rtok":
                nc.sync.dma_start(dbg[:, :], r_tok)
            elif tap == "qk":
                nc.sync.dma_start(dbg[:, :, :], qk_sb)
            elif tap == "v":
                nc.sync.dma_start(dbg[:, :, :, :], v_sb)
            elif tap == "avtok":
                nc.sync.dma_start(dbg[:, :, :], av_tok)
            elif tap == "avall":
                nc.sync.dma_start(dbg[:, :, :], av_all)
    nc.compile()
    return nc


def _fc_major(w):
    """[1024, 256] -> [2*128, 8*128]: row fc*128+p, cols kc*128+d (matches
    the [128, 2, KC, 128] SBUF layout loaded with a straight DMA)."""
    return np.ascontiguousarray(
        np.asarray(w, np.float64).reshape(KC, 128, 2, 128)
        .transpose(2, 1, 0, 3).reshape(2 * 128, KC * 128)
    ).astype(BF16)


def _host_inputs(x, norm_w, w_qkv, w_o, sin, cos):
    """Build the 8 per-core input maps (all bf16)."""
    n = T
    w_eff = np.asarray(w_qkv, np.float64) * np.asarray(norm_w, np.float64)[:, None]
    sin_n = np.asarray(sin, np.float32)[:n]  # [T, 64]
    cos_n = np.asarray(cos, np.float32)[:n]
    sign = np.concatenate([-np.ones(32, np.float32), np.ones(32, np.float32)])
    cos_tile = np.tile(cos_n.T, (2, 1))  # [128, T]
    sin_tile = np.tile((sin_n * sign[None, :]).T, (2, 1))  # [128, T]
    perm = np.zeros((128, 128), np.float32)
    for m in range(128):
        d = m % 64
        k = m + 32 if d < 32 else m - 32
        perm[k, m] = 1.0
    ident_np = np.eye(128, dtype=np.float32)
    ql = np.arange(128)[None, :]
    key = np.arange(128)[:, None]
    masks = (ql >= key).astype(np.float32)

    in_maps = []
    for c in range(8):
        b, g = c // 4, c % 4
        fs = slice(g * F, (g + 1) * F)
        xT_np = np.ascontiguousarray(np.asarray(x, np.float32)[b].T).astype(BF16)
        in_maps.append(
            {
                "xT": xT_np,
                "wq": _fc_major(w_eff[:, 0:DIM][:, fs] * (DIM_HEAD ** -0.5)),
                "wk": _fc_major(w_eff[:, DIM : 2 * DIM][:, fs]),
                "wv": w_eff[:, 2 * DIM : 3 * DIM][:, fs].astype(BF16),
                "wo": np.asarray(w_o, np.float32)[fs, :].astype(BF16),
                "cosT": cos_tile.astype(BF16),
                "sinT": sin_tile.astype(BF16),
                "perm": perm.astype(BF16),
                "masks": masks.astype(BF16),
                "ident": ident_np.astype(BF16),
            }
        )
    return in_maps


def kernel(x, norm_w, w_qkv, w_o, b_o, sin, cos):
    from concourse.bass_utils import run_bass_kernel_spmd

    if "nc" not in _NC_CACHE:
        _NC_CACHE["nc"] = _build_nc()
    nc = _NC_CACHE["nc"]
    in_maps = _host_inputs(x, norm_w, w_qkv, w_o, sin, cos)
    trace = bool(int(os.environ.get("KERNEL_TRACE", "0")))
    res = run_bass_kernel_spmd(nc, in_maps, core_ids=list(range(8)), trace=trace)
    if trace and res.exec_time_ns is not None:
        print(f"HW exec time: {res.exec_time_ns} ns")
    outs = [r["out"].astype(np.float32) for r in res.results]  # [1024, T] fm
    b_o = np.asarray(b_o, np.float32)
    full = np.empty((B, T, DIM), np.float32)
    for b in range(B):
        acc = outs[b * 4] + outs[b * 4 + 1] + outs[b * 4 + 2] + outs[b * 4 + 3]
        full[b] = acc.T + b_o[None, :]
    return full


# revision 94
# speedup vs baseline: 1.1261x; 1.0074x over previous
"""Trainium2 8-core kernel for RMSNorm -> QKV -> RoPE -> causal SDPA -> out-proj.

Sharding: core c = b*4 + g handles batch b (of 2) and heads 4g..4g+3 (of 16).
Each core computes a partial out-projection [dim, tokens]; the host sums the
4 head-group partials per batch (the tensor-parallel "unshard") and adds b_o.

Cost-model-driven layout (TimelineSim charges matmuls by OUTPUT FREE SIZE
only — contraction depth and output partitions are free):
  - scores per (head, kb): [key 128, q free] trimmed to the causal triangle.
  - AV runs TRANSPOSED: out [q 128, d 65] so each accumulation step costs 65
    rows instead of ~512; the ones column gives the softmax denominator.
    (matmul start=True zeroes the whole 2KB PSUM bank, so av4 tiles are
    bank-sized and only the first write uses start=True.)
  - The normalized token-major AV result returns to feature-major layout via
    PE transposes (53ns per 128x128 block) + one DVE copy per fc half.
  - exp for a head PAIR is fused into one Activation instruction (the two
    heads' score tiles sit in adjacent PSUM banks); Sqrt/Exp act tables are
    preloaded at t=0 with dummy activations.
  - r = rsqrt(mean x^2) rides into Q via r-scaled RoPE tables, into scores
    via the per-key `scale` operand of exp, and into V via a per-partition
    tensor_scalar during the PSUM->SBUF copy. r_tok (token-major r) comes
    from 16 free PE transposes of the r row.
  - Latency hiding: K/Q-fc0 project together chunk-paced off the DMA stream
    (PE idle gaps halve PE speed until 3us of continuous execution); the
    ss/r-chain, K/Q-fc1 and the tt0 RoPE interleave in wave 2; V projection,
    next-quarter RoPE, and the previous quarter's out-projection are all
    spread through the attention kb loops so the scalar engine (exp) stays
    fed; the last quarter's out-proj copies ride the then-idle Act engine.
"""

import os

import numpy as np
import ml_dtypes

BF16 = ml_dtypes.bfloat16

DIM = 1024
HEADS = 16
DIM_HEAD = 64
T = 2048  # tokens per batch
B = 2
HPC = 4  # heads per core
F = HPC * DIM_HEAD  # 256 per-core head width
KC = DIM // 128  # 8 contraction chunks
KORD = [2, 3, 4, 5, 6, 7, 0, 1]  # kc order: first matmul waits for chunk 2
TAIL_FINE = int(os.environ.get("KTAIL", "0"))
QT0_IN_RING = int(os.environ.get("KQT0", "0"))

_NC_CACHE = {}


def _build_nc():
    import concourse.bacc as bacc
    import concourse.mybir as mybir
    import concourse.tile as tile
    from contextlib import ExitStack

    f32 = mybir.dt.float32
    bf16 = mybir.dt.bfloat16
    nc = bacc.Bacc()

    xT = nc.declare_dram_parameter("xT", [DIM, T], bf16, isOutput=False)
    wq = nc.declare_dram_parameter("wq", [2 * 128, KC * 128], bf16, isOutput=False)
    wk = nc.declare_dram_parameter("wk", [2 * 128, KC * 128], bf16, isOutput=False)
    wv = nc.declare_dram_parameter("wv", [DIM, F], bf16, isOutput=False)
    wo = nc.declare_dram_parameter("wo", [F, DIM], bf16, isOutput=False)
    cosT = nc.declare_dram_parameter("cosT", [128, T], bf16, isOutput=False)
    sinT = nc.declare_dram_parameter("sinT", [128, T], bf16, isOutput=False)
    perm = nc.declare_dram_parameter("perm", [128, 128], bf16, isOutput=False)
    masks = nc.declare_dram_parameter("masks", [128, 128], bf16, isOutput=False)
    ident = nc.declare_dram_parameter("ident", [128, 128], bf16, isOutput=False)
    out = nc.declare_dram_parameter("out", [DIM, T], bf16, isOutput=True)
    tap = os.environ.get("KTAP", "")
    dbg = None
    if tap:
        _tap_shapes = {
            "rtok": ([128, 16], f32),
            "qk": ([128, 4, T], bf16),
            "v": ([128, 16, HPC, 65], bf16),
            "avtok": ([128, 16, F], bf16),
            "avall": ([128, 2, T], bf16),
        }
        shp, dt = _tap_shapes[tap]
        dbg = nc.declare_dram_parameter("dbg", shp, dt, isOutput=True)

    Exp = mybir.ActivationFunctionType.Exp
    Sqrt = mybir.ActivationFunctionType.Sqrt
    mult = mybir.AluOpType.mult
    add = mybir.AluOpType.add

    with ExitStack() as ctx:
        tc = ctx.enter_context(tile.TileContext(nc))
        consts = ctx.enter_context(tc.tile_pool(name="consts", bufs=1))
        persist = ctx.enter_context(tc.tile_pool(name="persist", bufs=1))
        work = ctx.enter_context(tc.tile_pool(name="work", bufs=4))
        vecs = ctx.enter_context(tc.tile_pool(name="vecs", bufs=1))

        # ---- constants / inputs ----
        wk_sb = consts.tile([128, 2, KC, 128], bf16, tag="wk")
        wq_sb = consts.tile([128, 2, KC, 128], bf16, tag="wq")
        wv_sb = consts.tile([128, KC, F], bf16, tag="wv")
        wo_sb = consts.tile([128, 2, DIM], bf16, tag="wo")
        cos_sb = consts.tile([128, T], bf16, tag="cos")
        sin_sb = consts.tile([128, T], bf16, tag="sin")
        perm_sb = consts.tile([128, 128], bf16, tag="perm")
        mask_sb = consts.tile([128, 128], bf16, tag="mask")
        id_sb = consts.tile([128, 128], bf16, tag="ident")
        ones_col = consts.tile([128, 1], bf16, tag="onesc")
        one_f32 = consts.tile([1, 1], f32, tag="onef")
        xT_sb = persist.tile([128, KC, T], bf16, tag="xT")
        xT_r = xT.rearrange("(kc p) t -> p kc t", p=128)
        # wk first (first PE consumer), then xT chunks in consumption order
        # with the other weights slotted behind the early chunks
        wk_r = wk.rearrange("(fc p) (kc d) -> p fc kc d", p=128, d=128)
        wq_r = wq.rearrange("(fc p) (kc d) -> p fc kc d", p=128, d=128)
        nc.sync.dma_start(wk_sb[:, 0], wk_r[:, 0])
        nc.sync.dma_start(wq_sb[:, 0], wq_r[:, 0])
        for kc in KORD[:4]:
            nc.sync.dma_start(xT_sb[:, kc], xT_r[:, kc])
        nc.sync.dma_start(wk_sb[:, 1], wk_r[:, 1])
        nc.sync.dma_start(wq_sb[:, 1], wq_r[:, 1])
        nc.sync.dma_start(perm_sb, perm[:, :])
        nc.sync.dma_start(cos_sb, cosT[:, :])
        nc.sync.dma_start(sin_sb, sinT[:, :])
        for kc in KORD[4:]:
            nc.sync.dma_start(xT_sb[:, kc], xT_r[:, kc])
        nc.sync.dma_start(wv_sb, wv.rearrange("(kc p) f -> p kc f", p=128))
        nc.sync.dma_start(mask_sb, masks[:, :])
        nc.sync.dma_start(id_sb, ident[:, :])
        nc.sync.dma_start(wo_sb, wo.rearrange("(fc p) d -> p fc d", p=128))
        nc.vector.memset(ones_col, 1.0)
        nc.vector.memset(one_f32, 1.0)

        # persistent activations
        qk_sb = persist.tile([128, 4, T], bf16, tag="qk")  # 0,1=q fc0/1; 2,3=k
        v_sb = persist.tile([128, 16, HPC, 65], bf16, tag="v")
        av_tok = persist.tile([128, 16, F], bf16, tag="avtok")
        av_all = persist.tile([128, 2, T], bf16, tag="av")
        r_sb = vecs.tile([1, T], f32, tag="r")
        r_tok = vecs.tile([128, 16], f32, tag="rtok")
        r_bc = persist.tile([128, T], f32, tag="rbc")
        cosr_sb = persist.tile([128, T], bf16, tag="cosr")
        sinr_sb = persist.tile([128, T], bf16, tag="sinr")
        qraw_sb = persist.tile([128, 2, 4, 512], bf16, tag="qraw")
        kraw_sb = persist.tile([128, 2, 4, 512], bf16, tag="kraw")
        nc.vector.memset(v_sb[:, :, :, 64:65], 1.0)
        # preload the Sqrt/Exp activation tables while DMAs stream in
        dum = vecs.tile([1, 1], f32, tag="dum")
        nc.scalar.activation(dum, one_f32, Sqrt)
        nc.scalar.activation(dum, dum, Exp)

        expp = ctx.enter_context(tc.tile_pool(name="expp", bufs=8))
        recp = ctx.enter_context(tc.tile_pool(name="recp", bufs=4))

        ctxA = ExitStack()
        psKQ = ctxA.enter_context(tc.tile_pool(name="psKQ", bufs=8, space="PSUM"))
        sbA = ctxA.enter_context(tc.tile_pool(name="sbA", bufs=1))
        xsq_sb = sbA.tile([128, KC, T], bf16, tag="xsq")

        # x^2 per chunk (DVE, chases the xT DMAs)
        for kc in KORD:
            nc.vector.tensor_mul(xsq_sb[:, kc], xT_sb[:, kc], xT_sb[:, kc])

        def rope_tt(fidx, tt, pool):
            """RoPE one 512-token slice of Q/K from the raw SBUF copy:
            rotate-half perm matmul + two multiplies + add into qk_sb.
            Q (fidx 0,1) uses the r-scaled tables so r_q rides in free."""
            ts = slice(tt * 512, (tt + 1) * 512)
            is_q = fidx < 2
            raw = (qraw_sb if is_q else kraw_sb)[:, fidx % 2, tt]
            cc = cosr_sb if is_q else cos_sb
            ssb = sinr_sb if is_q else sin_sb
            pp = pool.tile([128, 512], f32, tag="sc" if pool is not psKQ else "proj",
                           name=f"pp_{fidx}_{tt}")
            nc.tensor.matmul(pp, lhsT=perm_sb, rhs=raw, start=True, stop=True)
            t1 = work.tile([128, 512], bf16, tag="t1")
            nc.vector.tensor_tensor(t1, pp, ssb[:, ts], mult)
            t2 = work.tile([128, 512], bf16, tag="t2")
            nc.vector.tensor_tensor(t2, raw, cc[:, ts], mult)
            nc.vector.tensor_tensor(qk_sb[:, fidx, ts], t2, t1, add)

        # ---- wave 1: K-fc0 + Q-fc0 projections, chunk-paced off DMA ----
        psW = {}
        for nm in ("k0", "q0"):
            for tt in range(4):
                psW[(nm, tt)] = psKQ.tile(
                    [128, 512], f32, tag="proj", name=f"{nm}_{tt}"
                )
        for kc in KORD:
            for tt in range(4):
                ts = slice(tt * 512, (tt + 1) * 512)
                nc.tensor.matmul(
                    psW[("k0", tt)],
                    lhsT=wk_sb[:, 0, kc],
                    rhs=xT_sb[:, kc, ts],
                    start=(kc == KORD[0]),
                    stop=(kc == KORD[-1]),
                )
                nc.tensor.matmul(
                    psW[("q0", tt)],
                    lhsT=wq_sb[:, 0, kc],
                    rhs=xT_sb[:, kc, ts],
                    start=(kc == KORD[0]),
                    stop=(kc == KORD[-1]),
                )
        # free the k0 slots first (Act; DVE is still finishing x^2);
        # q0 copies are deferred into wave 2 so the r-chain starts sooner
        for tt in range(4):
            nc.scalar.copy(out=kraw_sb[:, 0, tt], in_=psW[("k0", tt)])

        # ---- wave 2: ss/r-chain + K-fc1 + Q-fc1 interleaved ----
        ss_sb = sbA.tile([1, T], f32, tag="ss")

        def proj_fc1(which, tt, pool=None):
            pool = pool if pool is not None else psKQ
            tg = "proj" if pool is psKQ else "sc"
            w = wk_sb if which == "k" else wq_sb
            psq = pool.tile([128, 512], f32, tag=tg, name=f"{which}1_{tt}")
            for kc in range(KC):
                nc.tensor.matmul(
                    psq,
                    lhsT=w[:, 1, kc],
                    rhs=xT_sb[:, kc, tt * 512 : (tt + 1) * 512],
                    start=(kc == 0),
                    stop=(kc == KC - 1),
                )
            if which == "k":
                nc.vector.tensor_copy(out=kraw_sb[:, 1, tt], in_=psq)
            else:
                nc.scalar.copy(out=qraw_sb[:, 1, tt], in_=psq)

        def ss_slice(s):
            ts = slice(s * 512, (s + 1) * 512)
            ss_ps = psKQ.tile([1, 512], f32, tag="proj", name=f"ss_{s}")
            for kc in range(KC):
                nc.tensor.matmul(
                    ss_ps,
                    lhsT=ones_col,
                    rhs=xsq_sb[:, kc, s * 512 : (s + 1) * 512],
                    start=(kc == 0),
                    stop=(kc == KC - 1),
                )
            nc.scalar.activation(
                ss_sb[:, ts], ss_ps, Sqrt, scale=1.0 / DIM
            )
            nc.vector.reciprocal(r_sb[:, ts], ss_sb[:, ts])
            nc.gpsimd.partition_broadcast(r_bc[:, ts], r_sb[:, ts])
            nc.gpsimd.tensor_tensor(cosr_sb[:, ts], cos_sb[:, ts], r_bc[:, ts], mult)
            nc.gpsimd.tensor_tensor(sinr_sb[:, ts], sin_sb[:, ts], r_bc[:, ts], mult)

        proj_fc1("q", 0)
        ss_slice(0)
        for tt in (0, 1):
            nc.scalar.copy(out=qraw_sb[:, 0, tt], in_=psW[("q0", tt)])
        rope_tt(2, 0, psKQ)
        proj_fc1("k", 0)
        ss_slice(1)
        rope_tt(0, 0, psKQ)
        for tt in (2, 3):
            nc.scalar.copy(out=qraw_sb[:, 0, tt], in_=psW[("q0", tt)])
        proj_fc1("q", 1)
        rope_tt(3, 0, psKQ)
        proj_fc1("k", 1)
        rope_tt(1, 0, psKQ)
        ss_slice(2)
        proj_fc1("q", 2)
        proj_fc1("k", 2)
        ss_slice(3)
        # r_tok via PE transposes of the r row
        rtok_ps = psKQ.tile([128, 16], f32, tag="proj", name="rtokps")
        for i in range(16):
            nc.tensor.transpose(
                rtok_ps[:, i : i + 1], r_sb[0:1, i * 128 : (i + 1) * 128],
                one_f32,
            )
        nc.vector.tensor_copy(out=r_tok, in_=rtok_ps)
        proj_fc1("q", 3)
        proj_fc1("k", 3)

        # ---- attention: scores [k,q] -> paired exp -> transposed AV ----
        # Quarter 0 runs INSIDE the psKQ ring (overlapping the QKV tail);
        # quarters 1-3 use dedicated pools: sc ring (3x2 banks, also V/pp/
        # outproj/avT) + av4 ring (2 banks).
        state = {"pending_oq": None, "sc_pool": psKQ, "split_sc": True,
                 "pending_proj": [("k", 2), ("q", 2), ("k", 3), ("q", 3)]}

        def v_proj(tt):
            pool = state["sc_pool"]
            tg = "proj" if pool is psKQ else "sc"
            psv = pool.tile([128, 256], f32, tag=tg, name=f"v_{tt}")
            for kc in range(KC):
                nc.tensor.matmul(
                    psv,
                    lhsT=xT_sb[:, kc, tt * 128 : (tt + 1) * 128],
                    rhs=wv_sb[:, kc, :],
                    start=(kc == 0),
                    stop=(kc == KC - 1),
                )
            nc.vector.tensor_scalar(
                out=v_sb[:, tt, :, 0:64],
                in0=psv.rearrange("p (h d) -> p h d", h=HPC),
                scalar1=r_tok[:, tt : tt + 1],
                scalar2=None,
                op0=mult,
            )

        def emit_outproj_do(qtp, do):
            pool = state["sc_pool"]
            tg = "proj" if pool is psKQ else "sc"
            po = pool.tile([128, 512], f32, tag=tg, name=f"o_{qtp}_{do}")
            for fc in range(2):
                nc.tensor.matmul(
                    po,
                    lhsT=wo_sb[:, fc, do * 128 : (do + 1) * 128],
                    rhs=av_all[:, fc, qtp * 512 : (qtp + 1) * 512],
                    start=(fc == 0),
                    stop=(fc == 1),
                )
            if qtp == 3:
                # stage into the dead kraw buffer; ship 2 merged DMAs so the
                # tail pays 2x625ns HWDGE instead of 8x; copies alternate
                # Act/DVE so the chain gating the DMA halves in length
                if do % 2 == 0:
                    nc.scalar.copy(out=kraw_sb[:, do // 4, do % 4, :], in_=po)
                else:
                    nc.vector.tensor_copy(
                        out=kraw_sb[:, do // 4, do % 4, :], in_=po
                    )
                if do % 4 == 3:
                    h = do // 4
                    nc.sync.dma_start(
                        out.rearrange("(do p) t -> p do t", p=128)[
                            :, h * 4 : h * 4 + 4, qtp * 512 : (qtp + 1) * 512
                        ],
                        kraw_sb[:, h],
                    )
            else:
                ob = work.tile([128, 512], bf16, tag="ob")
                nc.vector.tensor_copy(out=ob, in_=po)
                nc.sync.dma_start(
                    out.rearrange("(do p) t -> p do t", p=128)[
                        :, do, qtp * 512 : (qtp + 1) * 512
                    ],
                    ob,
                )

        def run_quarter(qt, av_pool):
            q0 = qt * 512
            pool = state["sc_pool"]
            tg = "proj" if pool is psKQ else "sc"
            split = state["split_sc"]
            for pi in range(2):
                # full-bank tiles: matmul start=True zeroes the whole 2KB
                # bank, so only the FIRST write into each bank uses it
                av4 = [
                    av_pool.tile(
                        [128, 4, 128], f32,
                        tag="proj" if av_pool is psKQ else "av4",
                        name=f"av_{qt}_{pi}_{x}",
                    )
                    for x in range(2)
                ]

                def emit_av(kb, ex):
                    for qbl in range(4):
                        qb = 4 * qt + qbl
                        if kb > qb:
                            continue
                        for x in range(2):
                            nc.tensor.matmul(
                                av4[x][:, qbl, 0:65],
                                lhsT=ex[:, x * 512 + qbl * 128 : x * 512 + (qbl + 1) * 128],
                                rhs=v_sb[:, kb, 2 * pi + x, :],
                                start=(kb == 0 and qbl == 0),
                                stop=(kb == qb),
                                skip_group_check=True,
                            )

                nkb = 4 * qt + 4
                pend = None
                for kb in range(nkb + 1):
                    cur = None
                    if kb < nkb:
                        c0 = max(0, kb * 128 - q0)
                        if split:
                            scs = [
                                pool.tile([128, 512], f32, tag=tg,
                                          name=f"sc_{qt}_{pi}_{kb}_{x}")
                                for x in range(2)
                            ]
                        else:
                            scp = pool.tile([128, 1024], f32, tag=tg,
                                            name=f"sc_{qt}_{pi}_{kb}")
                            scs = [scp[:, 0:512], scp[:, 512:1024]]
                        for x in range(2):
                            rX = slice(x * 64, x * 64 + 64)
                            nc.tensor.matmul(
                                scs[x][:, c0:512],
                                lhsT=qk_sb[rX, 2 + pi, kb * 128 : (kb + 1) * 128],
                                rhs=qk_sb[rX, pi, q0 + c0 : q0 + 512],
                                start=True,
                                stop=True,
                            )
                        if pi == 0 and kb < 4:
                            v_proj(4 * qt + kb)
                        # rope the NEXT quarter's token slice, one projection
                        # per kb iteration of pair 1
                        if pi == 1 and qt < 3 and kb < 4:
                            rope_tt((2, 0, 3, 1)[kb], qt + 1, pool)

                        if pi == 0 and state["pending_oq"] is not None and kb >= 2:
                            qtp, nd = state["pending_oq"]
                            todo = 8 - nd
                            left = nkb - kb
                            n_emit = -(-todo // max(left, 1))
                            for _ in range(min(n_emit, todo)):
                                emit_outproj_do(qtp, nd)
                                nd += 1
                            state["pending_oq"] = (qtp, nd) if nd < 8 else None
                        ex = expp.tile([128, 1024], bf16, tag="exp")
                        if split or c0 > 0:
                            for x in range(2):
                                nc.scalar.activation(
                                    ex[:, x * 512 + c0 : x * 512 + 512],
                                    scs[x][:, c0:512],
                                    Exp,
                                    scale=r_tok[:, kb : kb + 1],
                                )
                        else:
                            nc.scalar.activation(
                                ex, scp, Exp, scale=r_tok[:, kb : kb + 1]
                            )
                        if kb >= 4 * qt:  # diagonal block: causal mask
                            for x in range(2):
                                nc.gpsimd.tensor_tensor(
                                    ex[:, x * 512 + c0 : x * 512 + c0 + 128],
                                    ex[:, x * 512 + c0 : x * 512 + c0 + 128],
                                    mask_sb,
                                    mult,
                                )
                        cur = (kb, ex)
                    if pend is not None:
                        emit_av(*pend)
                    pend = cur
                # normalize (rows 0..63 / row 64) into token-major av_tok
                rec4s = []
                for x in range(2):
                    rec4 = recp.tile([128, 4], f32, tag="rec")
                    nc.vector.reciprocal(rec4, av4[x][:, :, 64:65])
                    rec4s.append(rec4)
                for qbl in range(4):
                    for x in range(2):
                        h = 2 * pi + x
                        nc.vector.tensor_scalar(
                            out=av_tok[:, 4 * qt + qbl, h * 64 : (h + 1) * 64],
                            in0=av4[x][:, qbl, 0:64],
                            scalar1=rec4s[x][:, qbl : qbl + 1],
                            scalar2=None,
                            op0=mult,
                        )
            # back to feature-major via PE transposes (53ns each)
            avT = pool.tile([128, 8, 128], bf16, tag=tg, name=f"avt_{qt}")
            for j, tt in enumerate(range(4 * qt, 4 * qt + 4)):
                for fc in range(2):
                    nc.tensor.transpose(
                        avT[:, fc * 4 + j, :],
                        av_tok[:, tt, fc * 128 : (fc + 1) * 128],
                        id_sb,
                    )
            for fc in range(2):
                nc.vector.tensor_copy(
                    out=av_all[:, fc, q0 : q0 + 512],
                    in_=avT[:, fc * 4 : fc * 4 + 4, :],
                )
            if qt < 3:
                # out-projection deferred into the next quarter's kb loop
                state["pending_oq"] = (qt, 0)
            else:
                for do in range(8):
                    emit_outproj_do(3, do)

        if QT0_IN_RING:
            # quarter 0 inside the psKQ ring, overlapping the QKV tail
            run_quarter(0, psKQ)
        ctxA.close()
        with (
            tc.tile_pool(name="psSC", bufs=3, space="PSUM") as psSC,
            tc.tile_pool(name="psAV", bufs=2, space="PSUM") as psAV,
        ):
            state["sc_pool"] = psSC
            state["split_sc"] = False
            for qt in range(0 if not QT0_IN_RING else 1, 4):
                run_quarter(qt, psAV)
            if tap == "rtok":
                nc.sync.dma_start(dbg[:, :], r_tok)
            elif tap == "qk":
                nc.sync.dma_start(dbg[:, :, :], qk_sb)
            elif tap == "v":
                nc.sync.dma_start(dbg[:, :, :, :], v_sb)
            elif tap == "avtok":
                nc.sync.dma_start(dbg[:, :, :], av_tok)
            elif tap == "avall":
                nc.sync.dma_start(dbg[:, :, :], av_all)
    nc.compile()
    return nc


def _fc_major(w):
    """[1024, 256] -> [2*128, 8*128]: row fc*128+p, cols kc*128+d (matches
    the [128, 2, KC, 128] SBUF layout loaded with a straight DMA)."""
    return np.ascontiguousarray(
        np.asarray(w, np.float64).reshape(KC, 128, 2, 128)
        .transpose(2, 1, 0, 3).reshape(2 * 128, KC * 128)
    ).astype(BF16)


def _host_inputs(x, norm_w, w_qkv, w_o, sin, cos):
    """Build the 8 per-core input maps (all bf16)."""
    n = T
    w_eff = np.asarray(w_qkv, np.float64) * np.asarray(norm_w, np.float64)[:, None]
    sin_n = np.asarray(sin, np.float32)[:n]  # [T, 64]
    cos_n = np.asarray(cos, np.float32)[:n]
    sign = np.concatenate([-np.ones(32, np.float32), np.ones(32, np.float32)])
    cos_tile = np.tile(cos_n.T, (2, 1))  # [128, T]
    sin_tile = np.tile((sin_n * sign[None, :]).T, (2, 1))  # [128, T]
    perm = np.zeros((128, 128), np.float32)
    for m in range(128):
        d = m % 64
        k = m + 32 if d < 32 else m - 32
        perm[k, m] = 1.0
    ident_np = np.eye(128, dtype=np.float32)
    ql = np.arange(128)[None, :]
    key = np.arange(128)[:, None]
    masks = (ql >= key).astype(np.float32)

    in_maps = []
    for c in range(8):
        b, g = c // 4, c % 4
        fs = slice(g * F, (g + 1) * F)
        xT_np = np.ascontiguousarray(np.asarray(x, np.float32)[b].T).astype(BF16)
        in_maps.append(
            {
                "xT": xT_np,
                "wq": _fc_major(w_eff[:, 0:DIM][:, fs] * (DIM_HEAD ** -0.5)),
                "wk": _fc_major(w_eff[:, DIM : 2 * DIM][:, fs]),
                "wv": w_eff[:, 2 * DIM : 3 * DIM][:, fs].astype(BF16),
                "wo": np.asarray(w_o, np.float32)[fs, :].astype(BF16),
                "cosT": cos_tile.astype(BF16),
                "sinT": sin_tile.astype(BF16),
                "perm": perm.astype(BF16),
                "masks": masks.astype(BF16),
                "ident": ident_np.astype(BF16),
            }
        )
    return in_maps


def kernel(x, norm_w, w_qkv, w_o, b_o, sin, cos):
    from concourse.bass_utils import run_bass_kernel_spmd

    if "nc" not in _NC_CACHE:
        _NC_CACHE["nc"] = _build_nc()
    nc = _NC_CACHE["nc"]
    in_maps = _host_inputs(x, norm_w, w_qkv, w_o, sin, cos)
    trace = bool(int(os.environ.get("KERNEL_TRACE", "0")))
    res = run_bass_kernel_spmd(nc, in_maps, core_ids=list(range(8)), trace=trace)
    if trace and res.exec_time_ns is not None:
        print(f"HW exec time: {res.exec_time_ns} ns")
    outs = [r["out"].astype(np.float32) for r in res.results]  # [1024, T] fm
    b_o = np.asarray(b_o, np.float32)
    full = np.empty((B, T, DIM), np.float32)
    for b in range(B):
        acc = outs[b * 4] + outs[b * 4 + 1] + outs[b * 4 + 2] + outs[b * 4 + 3]
        full[b] = acc.T + b_o[None, :]
    return full
